# revision 58
# baseline (speedup 1.0000x reference)
"""Trainium2 Bass kernel for a 3-layer ResGatedGraphConv GNN (ClinicalGatedGCN).

Strategy (8 NeuronCores, SPMD), v3:
  - Nodes partitioned into 8 contiguous ranges of 6250 (padded to 6272 = 49
    groups of 128). Edges assigned to the rank owning their dst node, sorted
    by (src-chunk epoch, dst-group, dst) on the host.
  - The [q|v] node table is built DISTRIBUTED: each rank computes q,v (and k)
    only for its OWN nodes from its SBUF-resident h (one fused matmul per
    128-node group against [Wq|Wv|Wk]), stores the [q|v] rows to local DRAM,
    and AllGathers the table in two chunks (groups 0:24 -> qv_t0, 24:49 ->
    qv_t1; both tables stay under the int16 gather-index limit). This removes
    the per-layer h AllGather, all h re-reads, and ~7/8 of the table matmuls
    and PSUM->SBUF copies of the replicated design.
  - One dma_gather per (epoch, dst-group) fetches the src rows of [q|v].
    k[dst] is expanded on the PE via a host-shipped 0/1 selector ST; the
    segment-sum over dst is a PE matmul against selector S. S and ST are
    packed in ONE fp8 blob per (epoch, group) (0/1 is exact in fp8; matmul
    operands may mix fp8 with bf16), halving selector DMA traffic.
  - The edge-embedding rank-1 term (attr x We + gate bias) is batched: one
    K=5 matmul per 4-tile PSUM chunk against a block-diagonal [5, 4H]
    constant, with per-chunk attr columns shipped once as an fp8 constant.
  - BatchNorm folding: A = gamma/sqrt(var+eps) is folded into the NEXT
    layer's weights (and the classifier); B is folded into effective biases.
    The on-device h update is a single fused leaky-ReLU
    (scalar_tensor_tensor max(x, slope*x)).
  - Mean-pool per graph is a matmul against a host-built indicator with
    1/cnt folded in; partial pools are AllGather'd and summed; the tiny
    classifier runs on every core.
"""

import numpy as np
import ml_dtypes

import concourse.bacc as bacc
import concourse.bass as bass
import concourse.mybir as mybir
import concourse.tile as tile
from concourse.bass_utils import run_bass_kernel_spmd
from concourse.masks import make_identity

F32 = mybir.dt.float32
BF16 = mybir.dt.bfloat16
FP8 = mybir.dt.float8e4
I16 = mybir.dt.int16
AF = mybir.ActivationFunctionType
OP = mybir.AluOpType

# ---------------- problem constants (hardcoded per spec) ----------------
N, E, H, G, NCLIN, NCLS = 50000, 800000, 128, 64, 16, 2
NLAYER = 3
EPS = 1e-5
SLOPE = 0.01
R = 8                      # ranks / NeuronCores

NPR = (N + R - 1) // R     # real nodes per rank (6250)
NGRP = (NPR + 127) // 128  # 128-node groups per rank (49)
NPAD = NGRP * 128          # padded nodes per rank (6272)
C0G = 24                   # groups in AG chunk 0
C1G = NGRP - C0G           # groups in AG chunk 1 (25)
C0N = C0G * 128            # 3072 rows
C1N = C1G * 128            # 3200 rows
KQC = 4                    # psum chunk size in edge tiles (1 PSUM bank)


def wrap_idxs_block(idx):
    """Wrap one gather call's indices: idx j -> [j%16, j//16], tiled to 128 parts."""
    n = len(idx)
    assert n % 16 == 0
    w = np.asarray(idx, np.int16).reshape(n // 16, 16).T
    return np.tile(w, (8, 1))


# ---------------------------------------------------------------------------
# host-side preprocessing
# ---------------------------------------------------------------------------

def prep(inputs):
    x = np.asarray(inputs["x"], np.float32)
    edge_index = np.asarray(inputs["edge_index"])
    edge_attr = np.asarray(inputs["edge_attr"], np.float32)[:, 0]
    batch = np.asarray(inputs["batch"]).astype(np.int64)
    clinical = np.asarray(inputs["clinical"], np.float32)
    Wk, bk = np.asarray(inputs["Wk"], np.float32), np.asarray(inputs["bk"], np.float32)
    Wq, bq = np.asarray(inputs["Wq"], np.float32), np.asarray(inputs["bq"], np.float32)
    Wv, bv = np.asarray(inputs["Wv"], np.float32), np.asarray(inputs["bv"], np.float32)
    Ws, bs = np.asarray(inputs["Ws"], np.float32), np.asarray(inputs["bs"], np.float32)
    We, be = np.asarray(inputs["We"], np.float32), np.asarray(inputs["be"], np.float32)
    gamma = np.asarray(inputs["gamma"], np.float32)
    beta = np.asarray(inputs["beta"], np.float32)
    rmean = np.asarray(inputs["rmean"], np.float32)
    rvar = np.asarray(inputs["rvar"], np.float32)
    Wc, bc = np.asarray(inputs["Wc"], np.float32), np.asarray(inputs["bc"], np.float32)

    src = edge_index[0].astype(np.int64)
    dst = edge_index[1].astype(np.int64)

    # BN folded: true h_out = A*leaky(pre) + B; device h3 = leaky(pre).
    A = gamma / np.sqrt(rvar + EPS)          # [3, H]
    B = beta - rmean * A                     # [3, H]

    # effective weights: fold diag(A[l-1]) into layer-l input maps, and the
    # B[l-1] offset into layer-l biases.
    Wq_e = np.stack([Wq[l] * (A[l - 1][:, None] if l else 1.0) for l in range(NLAYER)])
    Wk_e = np.stack([Wk[l] * (A[l - 1][:, None] if l else 1.0) for l in range(NLAYER)])
    Wv_e = np.stack([Wv[l] * (A[l - 1][:, None] if l else 1.0) for l in range(NLAYER)])
    Ws_e = np.stack([Ws[l] * (A[l - 1][:, None] if l else 1.0) for l in range(NLAYER)])
    bgate = np.stack([bk[l] + bq[l] + be[l]
                      + (B[l - 1] @ (Wk[l] + Wq[l]) if l else 0.0)
                      for l in range(NLAYER)])
    bv_e = np.stack([bv[l] + (B[l - 1] @ Wv[l] if l else 0.0) for l in range(NLAYER)])
    bs_e = np.stack([bs[l] + (B[l - 1] @ Ws[l] if l else 0.0) for l in range(NLAYER)])
    Wc_h = Wc[0:H] * A[2][:, None]
    bc_e = bc + B[2] @ Wc[0:H]

    Wqvk = np.concatenate([Wq_e, Wv_e, Wk_e], axis=2)     # [3, H, 3H]

    # block-diagonal edge-term constant: rows 0..3 carry We on diag block,
    # row 4 carries the gate bias (broadcast to each tile block).
    BD = np.zeros((NLAYER, 5, KQC * H), np.float32)
    for l in range(NLAYER):
        for t in range(KQC):
            BD[l, t, t * H:(t + 1) * H] = We[l, 0, :]
            BD[l, 4, t * H:(t + 1) * H] = bgate[l]

    has_bv = bool(np.any(bv_e != 0))
    has_bs = bool(np.any(bs_e != 0))

    # ---- edge structure ----
    e_rank = dst // NPR
    dst_local = dst - e_rank * NPR
    grp = dst_local // 128
    drel = dst_local % 128
    rs = src // NPR
    lr = src - rs * NPR
    ep = (lr >= C0N).astype(np.int64)
    trow = np.where(ep == 0, rs * C0N + lr, rs * C1N + (lr - C0N))

    counts = np.zeros((2, R, NGRP), np.int64)
    np.add.at(counts, (ep, e_rank, grp), 1)
    nt_l = [np.ceil(counts[e].max(axis=0) / 128).astype(int) for e in (0, 1)]
    off_l = [np.concatenate([[0], np.cumsum(nt)]).astype(int) for nt in nt_l]
    nch_l = [np.ceil(nt / KQC).astype(int) for nt in nt_l]
    choff_l = [np.concatenate([[0], np.cumsum(nc)]).astype(int) for nc in nch_l]

    cntg = np.bincount(batch, minlength=G).astype(np.float32)
    inv_cnt = 1.0 / np.maximum(cntg, 1.0)

    order = np.lexsort((dst, grp, ep, e_rank))
    trow_s, drel_s, attr_s = trow[order], drel[order], edge_attr[order]
    key = (e_rank[order] * 2 + ep[order]) * NGRP + grp[order]
    starts = np.searchsorted(key, np.arange(R * 2 * NGRP + 1))

    in_maps = []
    for r in range(R):
        ep_arrs = {}
        for e in (0, 1):
            nt = nt_l[e]
            off = off_l[e]
            nch = nch_l[e]
            choff = choff_l[e]
            tot = int(off[-1])
            chtot = int(choff[-1])
            gidx = np.zeros((128, tot * 8), np.int16)
            blob = np.zeros((128, tot * 256), ml_dtypes.float8_e4m3)
            attr4 = np.zeros((5, chtot * 128), ml_dtypes.float8_e4m3)
            attr4[4, :] = 1.0
            for g in range(NGRP):
                ntg = int(nt[g])
                if ntg == 0:
                    continue
                k = (r * 2 + e) * NGRP + g
                s0 = int(starts[k])
                n = int(counts[e, r, g])
                o = int(off[g])
                co = int(choff[g])
                idx = np.full((ntg * 128,), -1, np.int64)
                idx[:n] = trow_s[s0:s0 + n]
                if n == 0:
                    idx[0] = 0
                gidx[:, o * 8:(o + ntg) * 8] = wrap_idxs_block(idx)
                j = np.arange(n)
                t = j // 128
                p = j % 128
                dr = drel_s[s0:s0 + n].astype(np.int64)
                # S: [p_edge, d] at cols (o+t)*256 + d
                blob[p, (o + t) * 256 + dr] = 1
                # ST: [drel, p_edge] at cols (o+t)*256 + 128 + p
                blob[dr, (o + t) * 256 + 128 + p] = 1
                attr4[t % KQC, (co + t // KQC) * 128 + p] = \
                    attr_s[s0:s0 + n].astype(ml_dtypes.float8_e4m3)
            ep_arrs[e] = (gidx, blob, attr4)
        IndT = np.zeros((NPAD, G), np.float32)
        lo, hi = r * NPR, min((r + 1) * NPR, N)
        IndT[np.arange(hi - lo), batch[lo:hi]] = inv_cnt[batch[lo:hi]]
        xT_loc = np.zeros((128, NPAD), np.float32)
        xT_loc[:, 0:hi - lo] = x[lo:hi].T
        x_rb = np.zeros((R * 128, NPAD), np.float32)
        for rr in range(R):
            rlo, rhi = rr * NPR, min((rr + 1) * NPR, N)
            x_rb[rr * 128:(rr + 1) * 128, 0:rhi - rlo] = x[rlo:rhi].T
        gcnt = np.maximum(counts[:, r, :], 1).astype(np.int32).reshape(1, 2 * NGRP)
        im = {
            "gcnt": gcnt,
            "xT_loc": xT_loc.astype(ml_dtypes.float8_e4m3),
            "x_rb": x_rb.astype(ml_dtypes.float8_e4m3),
            "Wqvk": Wqvk.astype(ml_dtypes.bfloat16),
            "Ws": Ws_e.astype(ml_dtypes.bfloat16),
            "BD": BD.astype(ml_dtypes.bfloat16),
            "bv_row": np.concatenate(
                [np.zeros((NLAYER, 1, H), np.float32),
                 bv_e.reshape(NLAYER, 1, H),
                 np.zeros((NLAYER, 1, H), np.float32)], axis=2
            ).astype(ml_dtypes.bfloat16),
            "bs_col": bs_e.reshape(NLAYER, H, 1),
            "gidx0": ep_arrs[0][0], "blob0": ep_arrs[0][1], "attr0": ep_arrs[0][2],
            "gidx1": ep_arrs[1][0], "blob1": ep_arrs[1][1], "attr1": ep_arrs[1][2],
            "IndT": IndT.astype(ml_dtypes.bfloat16),
            "clinT": clinical.T.copy(),
            "Wc_h": Wc_h, "Wc_c": Wc[H:H + NCLIN],
            "bc_rep": np.tile(bc_e, (G, 1)),
        }
        in_maps.append(im)
    meta = dict(NT0=tuple(int(v) for v in nt_l[0]),
                NT1=tuple(int(v) for v in nt_l[1]),
                has_bv=has_bv, has_bs=has_bs, tab_fp8=True, dr_seg=True)
    return in_maps, meta


# ---------------------------------------------------------------------------
# device program
# ---------------------------------------------------------------------------

def build(meta):
    parts = meta.get("parts", 4)
    DT = BF16
    TDT = FP8 if meta.get("tab_fp8", True) else BF16
    NT_L = [list(meta["NT0"]), list(meta["NT1"])]
    OFF_L = [np.concatenate([[0], np.cumsum(nt)]).astype(int) for nt in NT_L]
    NCH_L = [np.ceil(np.array(nt) / KQC).astype(int) for nt in NT_L]
    CHOFF_L = [np.concatenate([[0], np.cumsum(nc)]).astype(int) for nc in NCH_L]
    TOT = [int(o[-1]) for o in OFF_L]
    CHTOT = [int(c[-1]) for c in CHOFF_L]
    TMAX = max(max(NT_L[0]), max(NT_L[1]))

    nc = bacc.Bacc("TRN2", target_bir_lowering=False, debug=False, num_devices=R)

    def din(name, shape, dt):
        return nc.dram_tensor(name, shape, dt, kind="ExternalInput").ap()

    t_xT = din("xT_loc", [128, NPAD], FP8)
    t_cnt = din("gcnt", [1, 2 * NGRP], mybir.dt.int32)
    t_xrb = din("x_rb", [R * 128, NPAD], FP8)
    t_Wqvk = din("Wqvk", [NLAYER, H, 3 * H], DT)
    t_Ws = din("Ws", [NLAYER, H, H], DT)
    t_BD = din("BD", [NLAYER, 5, KQC * H], DT)
    t_bv = din("bv_row", [NLAYER, 1, 3 * H], DT)
    t_bs = din("bs_col", [NLAYER, H, 1], F32)
    t_gidx = [din("gidx0", [128, TOT[0] * 8], I16),
              din("gidx1", [128, TOT[1] * 8], I16)]
    t_blob = [din("blob0", [128, TOT[0] * 256], FP8),
              din("blob1", [128, TOT[1] * 256], FP8)]
    t_attr = [din("attr0", [5, CHTOT[0] * 128], FP8),
              din("attr1", [5, CHTOT[1] * 128], FP8)]
    t_IndT = din("IndT", [NPAD, G], DT)
    t_clinT = din("clinT", [NCLIN, G], F32)
    t_Wc_h = din("Wc_h", [H, NCLS], F32)
    t_Wc_c = din("Wc_c", [NCLIN, NCLS], F32)
    t_bc = din("bc_rep", [G, NCLS], F32)

    t_out = nc.dram_tensor("out", [G, NCLS], F32, kind="ExternalOutput").ap()

    qv_loc = nc.dram_tensor("qv_loc", [NPAD, 2 * H], TDT).ap()
    # per-layer AG'd table pairs (separate per layer so the next layer's
    # AllGather never overwrites a table the current layer still gathers from)
    qv_tl = [None,
             [nc.dram_tensor("qv1_t0", [R * C0N, 2 * H], TDT, addr_space="Shared").ap(),
              nc.dram_tensor("qv1_t1", [R * C1N, 2 * H], TDT, addr_space="Shared").ap()],
             [nc.dram_tensor("qv2_t0", [R * C0N, 2 * H], TDT, addr_space="Shared").ap(),
              nc.dram_tensor("qv2_t1", [R * C1N, 2 * H], TDT, addr_space="Shared").ap()]]
    # layer-0 tables are built locally (x is replicated), no collective
    qv_tl[0] = [nc.dram_tensor("qv0_t0", [R * C0N, 2 * H], TDT).ap(),
                nc.dram_tensor("qv0_t1", [R * C1N, 2 * H], TDT).ap()]
    pool_in = nc.dram_tensor("pool_in", [G, H], F32).ap()
    pool_out = nc.dram_tensor("pool_out", [R * G, H], F32, addr_space="Shared").ap()

    with tile.TileContext(nc) as tc:
        import contextlib
        with contextlib.ExitStack() as ctx:
            consts = ctx.enter_context(tc.tile_pool(name="consts", bufs=1))
            hsb = ctx.enter_context(tc.tile_pool(name="hsb", bufs=1))
            h3p = ctx.enter_context(tc.tile_pool(name="h3p", bufs=1))
            ksb = ctx.enter_context(tc.tile_pool(name="ksb", bufs=1))
            stg = ctx.enter_context(tc.tile_pool(name="stg", bufs=4))
            tail = ctx.enter_context(tc.tile_pool(name="tail", bufs=2))
            edg = ctx.enter_context(tc.tile_pool(name="edg", bufs=5))
            sel = ctx.enter_context(tc.tile_pool(name="sel", bufs=8))
            edm = ctx.enter_context(tc.tile_pool(name="edm", bufs=3))
            pnode = ctx.enter_context(tc.tile_pool(name="pnode", bufs=2, space="PSUM"))
            pkq = ctx.enter_context(tc.tile_pool(name="pkq", bufs=2, space="PSUM"))
            pedge = ctx.enter_context(tc.tile_pool(name="pedge", bufs=2, space="PSUM"))
            ppool = ctx.enter_context(tc.tile_pool(name="ppool", bufs=1, space="PSUM"))

            _cid = [0]

            def load_const(src_ap, shape, dt):
                _cid[0] += 1
                t = consts.tile(shape, dt, tag=f"c{_cid[0]}_{src_ap.tensor.name}")
                nc.sync.dma_start(t[:], src_ap)
                return t

            Wqvk_t = [load_const(t_Wqvk[l], [H, 3 * H], DT) for l in range(NLAYER)]
            Ws_t = [load_const(t_Ws[l], [H, H], DT) for l in range(NLAYER)]
            BD_t = [load_const(t_BD[l], [5, KQC * H], DT) for l in range(NLAYER)]
            bv_t = ([load_const(t_bv[l], [1, 3 * H], DT) for l in range(NLAYER)]
                    if meta["has_bv"] else None)
            bs_t = ([load_const(t_bs[l], [H, 1], F32) for l in range(NLAYER)]
                    if meta["has_bs"] else None)
            ones_t = consts.tile([1, 128], DT)
            nc.vector.memset(ones_t[:], 1.0)
            gidx_t = [load_const(t_gidx[0], [128, TOT[0] * 8], I16),
                      load_const(t_gidx[1], [128, TOT[1] * 8], I16)]
            attr_t = [load_const(t_attr[0], [5, CHTOT[0] * 128], FP8),
                      load_const(t_attr[1], [5, CHTOT[1] * 128], FP8)]
            xT_t = load_const(t_xT, [128, NPAD], FP8)
            cnt_t = load_const(t_cnt, [1, 2 * NGRP], mybir.dt.int32)
            ident = consts.tile([128, 128], DT)
            make_identity(nc, ident[:])
            identf = consts.tile([128, 128], F32)
            make_identity(nc, identf[:])
            clin_t = load_const(t_clinT, [NCLIN, G], F32)
            Wch_t = load_const(t_Wc_h, [H, NCLS], F32)
            Wcc_t = load_const(t_Wc_c, [NCLIN, NCLS], F32)
            bc_t = load_const(t_bc, [G, NCLS], F32)

            # ping-pong state by layer parity
            hs_pp = [hsb.tile([128, NPAD], DT, tag="hs0", name="hs0"),
                     hsb.tile([128, NPAD], DT, tag="hs1", name="hs1")]
            k_pp = [ksb.tile([128, NGRP, H], DT, tag="k0", name="k0"),
                    ksb.tile([128, NGRP, H], DT, tag="k1", name="k1")]
            h3_pp = [h3p.tile([128, NPAD], DT, tag="h3a", name="h3a"),
                     h3p.tile([128, NPAD], DT, tag="h3b", name="h3b")]
            h3f = h3_pp[0]                             # layer-2 output parity

            def build_chunk(l, c, hsrc):
                """Local q|v|k for groups of AG-chunk c of layer l."""
                g0, g1 = (0, C0G) if c == 0 else (C0G, NGRP)
                k_sb = k_pp[l % 2]
                g = g0
                while g < g1:
                    ns = min(8, g1 - g)
                    st = stg.tile([128, 8, 2 * H], TDT, tag="st")
                    for s in range(ns):
                        gg = g + s
                        ps = pnode.tile([128, 512], F32, tag="pn")
                        nc.tensor.matmul(
                            out=ps[:, 0:3 * H],
                            lhsT=hsrc[:, gg * 128:(gg + 1) * 128],
                            rhs=Wqvk_t[l][:], start=True,
                            stop=not meta["has_bv"],
                            skip_group_check=True)
                        if meta["has_bv"]:
                            nc.tensor.matmul(
                                out=ps[:, 0:3 * H], lhsT=ones_t[:],
                                rhs=bv_t[l][:], start=False, stop=True,
                                skip_group_check=True)
                        nc.scalar.activation(st[:, s, :], ps[:, 0:2 * H],
                                             AF.Copy)
                        nc.scalar.activation(k_sb[:, gg, :],
                                             ps[:, 2 * H:3 * H], AF.Copy)
                    nc.sync.dma_start(
                        qv_loc[g * 128:(g + ns) * 128, :].rearrange(
                            "(s p) f -> p s f", p=128),
                        st[:, 0:ns, :])
                    g += ns

            def emit_ag(l, c):
                ins = qv_loc[0:C0N, :] if c == 0 else qv_loc[C0N:NPAD, :]
                nc.gpsimd.collective_compute(
                    "AllGather", OP.bypass, replica_groups=[list(range(R))],
                    ins=[ins], outs=[qv_tl[l][c][:]])

            def emit_spart(l, hsrc):
                hs = hs_pp[l % 2]
                for c0 in range(0, NPAD, 512):
                    csz = min(512, NPAD - c0)
                    pss = pnode.tile([128, 512], F32, tag="pn")
                    nc.tensor.matmul(out=pss[:, 0:csz], lhsT=Ws_t[l][:],
                                     rhs=hsrc[:, c0:c0 + csz],
                                     start=True, stop=True)
                    if meta["has_bs"]:
                        nc.scalar.activation(hs[:, c0:c0 + csz], pss[:, 0:csz],
                                             AF.Identity, bias=bs_t[l][:],
                                             scale=1.0)
                    else:
                        nc.scalar.activation(hs[:, c0:c0 + csz], pss[:, 0:csz],
                                             AF.Copy)

            def make_tail_cb(l, c_start, c_end):
                # During layer l's ep1 sweep over [c_start, c_end): as each
                # dst-group finalizes, leaky just that slice and build the
                # next layer's table rows for it, storing every 8 groups so
                # the chunk AllGather can fire the moment the sweep ends.
                st_state = {}

                def cb(g, l=l):
                    sl = slice(g * 128, (g + 1) * 128)
                    nc.vector.scalar_tensor_tensor(
                        out=h3_pp[l % 2][:, sl], in0=hs_pp[l % 2][:, sl],
                        scalar=float(SLOPE), in1=hs_pp[l % 2][:, sl],
                        op0=OP.mult, op1=OP.max)
                    if l >= NLAYER - 1:
                        return
                    s = (g - c_start) % 8
                    if s == 0:
                        st_state["st"] = stg.tile([128, 8, 2 * H], TDT,
                                                  tag="st", name="st_cb")
                        st_state["g0"] = g
                    st = st_state["st"]
                    ps = pnode.tile([128, 512], F32, tag="pn", name="pn_cb")
                    nc.tensor.matmul(
                        out=ps[:, 0:3 * H],
                        lhsT=h3_pp[l % 2][:, sl],
                        rhs=Wqvk_t[l + 1][:], start=True,
                        stop=not meta["has_bv"], skip_group_check=True)
                    if meta["has_bv"]:
                        nc.tensor.matmul(
                            out=ps[:, 0:3 * H], lhsT=ones_t[:],
                            rhs=bv_t[l + 1][:], start=False, stop=True,
                            skip_group_check=True)
                    nc.scalar.activation(st[:, s, :], ps[:, 0:2 * H], AF.Copy)
                    nc.scalar.activation(k_pp[(l + 1) % 2][:, g, :],
                                         ps[:, 2 * H:3 * H], AF.Copy)
                    if s == 7 or g == c_end - 1:
                        g0b = st_state["g0"]
                        nc.sync.dma_start(
                            qv_loc[g0b * 128:(g + 1) * 128, :].rearrange(
                                "(s p) f -> p s f", p=128),
                            st[:, 0:g + 1 - g0b, :])
                return cb

            def emit_leaky(l, c):
                sl = slice(0, C0N) if c == 0 else slice(C0N, NPAD)
                nc.vector.scalar_tensor_tensor(
                    out=h3_pp[l % 2][:, sl], in0=hs_pp[l % 2][:, sl],
                    scalar=float(SLOPE), in1=hs_pp[l % 2][:, sl],
                    op0=OP.mult, op1=OP.max)

            def emit_edges(e, l, g0, g1, after_group=None):
                nt_list = NT_L[e]
                off = OFF_L[e]
                choff = CHOFF_L[e]
                hs = hs_pp[l % 2]
                k_sb = k_pp[l % 2]
                tab = qv_tl[l][e]
                for g in range(g0, g1):
                    nt = int(nt_list[g])
                    if nt == 0:
                        if after_group is not None:
                            after_group(g)
                        continue
                    ne = nt * 128
                    o = int(off[g])
                    co = int(choff[g])
                    bt = sel.tile([128, TMAX, 256], FP8, tag="blob")
                    nc.sync.dma_start(
                        bt[:, 0:nt, :],
                        t_blob[e][:, o * 256:(o + nt) * 256].rearrange(
                            "p (t d) -> p t d", t=nt))
                    gt = edg.tile([128, TMAX, 2 * H], TDT, tag="g")
                    nreg = nc.gpsimd.value_load(
                        cnt_t[0:1, e * NGRP + g:e * NGRP + g + 1])
                    nc.gpsimd.dma_gather(
                        gt[:, 0:nt, :], tab[:],
                        gidx_t[e][:, o * 8:(o + nt) * 8],
                        ne, nreg, 2 * H, single_packet=(ne <= 512))
                    ktb = edm.tile([128, TMAX, 128], DT, tag="kt")
                    for ci in range((nt + KQC - 1) // KQC):
                        q0 = ci * KQC
                        qn = min(KQC, nt - q0)
                        pk = pkq.tile([128, KQC, 128], F32, tag="kq")
                        nc.tensor.matmul(
                            out=pk[:, 0:KQC, :],
                            lhsT=attr_t[e][0:5, (co + ci) * 128:(co + ci + 1) * 128],
                            rhs=BD_t[l][:], start=True, stop=False,
                            skip_group_check=True)
                        for t in range(q0, q0 + qn):
                            nc.tensor.matmul(
                                out=pk[:, t - q0, :],
                                lhsT=bt[:, t, 128:256],
                                rhs=k_sb[:, g, :], start=False, stop=False,
                                skip_group_check=True)
                        # q[src] folded into the same PSUM via identity matmul
                        nc.tensor.matmul(
                            out=pk[:, 0:qn, :], lhsT=ident[:],
                            rhs=gt[:, q0:q0 + qn, 0:H], start=False, stop=True,
                            skip_group_check=True)
                        nc.scalar.activation(ktb[:, q0:q0 + qn, :],
                                             pk[:, 0:qn, :], AF.Sigmoid)
                    dr = meta.get("dr_seg", False)
                    msg = edm.tile([128, TMAX, 128], FP8 if dr else DT, tag="msg")
                    nc.vector.tensor_tensor(out=msg[:, 0:nt, :],
                                            in0=ktb[:, 0:nt, :],
                                            in1=gt[:, 0:nt, H:2 * H], op=OP.mult)
                    pa = pedge.tile([128, 128], F32, tag="pa")
                    if dr:
                        npair = nt // 2
                        for t2 in range(0, npair * 2, 2):
                            nc.tensor.matmul(
                                out=pa[:], lhsT=msg[:, t2:t2 + 2, :],
                                rhs=bt[:, t2:t2 + 2, 0:128], start=(t2 == 0),
                                stop=(t2 + 2 == nt),
                                perf_mode=mybir.MatmulPerfMode.DoubleRow,
                                skip_group_check=True)
                        if nt % 2:
                            nc.tensor.matmul(out=pa[:], lhsT=msg[:, nt - 1, :],
                                             rhs=bt[:, nt - 1, 0:128],
                                             start=(nt == 1), stop=True,
                                             skip_group_check=True)
                    else:
                        for t in range(nt):
                            nc.tensor.matmul(out=pa[:], lhsT=msg[:, t, :],
                                             rhs=bt[:, t, 0:128], start=(t == 0),
                                             stop=(t == nt - 1))
                    nc.vector.tensor_tensor(
                        out=hs[:, g * 128:(g + 1) * 128],
                        in0=hs[:, g * 128:(g + 1) * 128], in1=pa[:], op=OP.add)
                    if after_group is not None:
                        after_group(g)

            # ---- layer-0 table from x: replicated input, so each rank builds
            # the FULL table locally (no collective, no exposed startup AG).
            def build_l0_chunk(c):
                g0, g1, tab, cbase = ((0, C0G, qv_tl[0][0], C0N) if c == 0
                                      else (C0G, NGRP, qv_tl[0][1], C1N))
                for rb in range(R):
                    g = g0
                    while g < g1:
                        ns = min(8, g1 - g)
                        lh = stg.tile([128, 1024], FP8, tag="lh")
                        nc.sync.dma_start(
                            lh[:, 0:ns * 128],
                            t_xrb[rb * 128:(rb + 1) * 128,
                                  g * 128:(g + ns) * 128])
                        st = stg.tile([128, 8, 2 * H], TDT, tag="st")
                        for s in range(0, ns, 2):
                            n2 = min(2, ns - s)
                            ps = pnode.tile([128, 512], F32, tag="pn")
                            for u in range(n2):
                                nc.tensor.matmul(
                                    out=ps[:, u * 256:u * 256 + 2 * H],
                                    lhsT=lh[:, (s + u) * 128:(s + u + 1) * 128],
                                    rhs=Wqvk_t[0][:, 0:2 * H], start=True,
                                    stop=not meta["has_bv"],
                                    skip_group_check=True)
                                if meta["has_bv"]:
                                    nc.tensor.matmul(
                                        out=ps[:, u * 256:u * 256 + 2 * H],
                                        lhsT=ones_t[:],
                                        rhs=bv_t[0][0:1, 0:2 * H],
                                        start=False, stop=True,
                                        skip_group_check=True)
                            if (s // 2) % 2 == 0:
                                nc.scalar.activation(st[:, s:s + n2, :],
                                                     ps[:, 0:n2 * 256], AF.Copy)
                            else:
                                nc.vector.tensor_copy(st[:, s:s + n2, :],
                                                      ps[:, 0:n2 * 256])
                        r0 = rb * cbase + (g - g0) * 128
                        nc.sync.dma_start(
                            tab[r0:r0 + ns * 128, :].rearrange(
                                "(s p) f -> p s f", p=128),
                            st[:, 0:ns, :])
                        g += ns
            build_l0_chunk(0)
            # local k table + s-part for layer 0 from the local x slice
            for g in range(NGRP):
                psk = pnode.tile([128, 512], F32, tag="pn")
                nc.tensor.matmul(out=psk[:, 0:H],
                                 lhsT=xT_t[:, g * 128:(g + 1) * 128],
                                 rhs=Wqvk_t[0][:, 2 * H:3 * H],
                                 start=True, stop=True, skip_group_check=True)
                nc.scalar.activation(k_pp[0][:, g, :], psk[:, 0:H], AF.Copy)
            emit_spart(0, xT_t)

            for _ in range(5):
                gz = edg.tile([128, TMAX, 2 * H], TDT, tag="g")
                nc.vector.memset(gz[:], 0.0)

            pp = ppool.tile([G, H], F32)
            indc = consts.tile([128, NGRP, G], DT)
            nc.sync.dma_start(
                indc[:], t_IndT[:].rearrange("(c p) g -> p c g", p=128))

            def make_pool_cb(l):
                def cb(g, l=l):
                    sl = slice(g * 128, (g + 1) * 128)
                    nc.vector.scalar_tensor_tensor(
                        out=h3_pp[l % 2][:, sl], in0=hs_pp[l % 2][:, sl],
                        scalar=float(SLOPE), in1=hs_pp[l % 2][:, sl],
                        op0=OP.mult, op1=OP.max)
                    trp = pedge.tile([128, 128], DT, tag="pa", name="trp_cb")
                    nc.tensor.transpose(out=trp[:], in_=h3f[:, sl],
                                        identity=ident[:])
                    hnode = tail.tile([128, 128], DT, tag="hnode",
                                      name="hnode_cb")
                    nc.vector.tensor_copy(hnode[:], trp[:])
                    nc.tensor.matmul(out=pp[:], lhsT=indc[:, g, :],
                                     rhs=hnode[:],
                                     start=(g == 0), stop=(g == NGRP - 1))
                return cb

            def emit_pool_part(c0g, c1g):
                for c in range(c0g, c1g):
                    trp = pedge.tile([128, 128], DT, tag="pa")
                    nc.tensor.transpose(out=trp[:],
                                        in_=h3f[:, c * 128:(c + 1) * 128],
                                        identity=ident[:])
                    hnode = tail.tile([128, 128], DT, tag="hnode")
                    nc.vector.tensor_copy(hnode[:], trp[:])
                    nc.tensor.matmul(out=pp[:],
                                     lhsT=indc[:, c, :],
                                     rhs=hnode[:],
                                     start=(c == 0), stop=(c == NGRP - 1))

            for l in range(NLAYER):
                hsrc_next = h3_pp[l % 2]
                last = l >= NLAYER - 1
                if parts >= 3 and not last:
                    cb0 = make_tail_cb(l, 0, C0G)
                    cb1 = make_tail_cb(l, C0G, NGRP)
                elif parts >= 4 and last:
                    cb0 = cb1 = make_pool_cb(l)
                else:
                    cb0 = cb1 = None
                if parts >= 2:
                    if l == 0:
                        # layer 0 consumes no AllGather (local tables), so run
                        # both epochs' first halves up front and fire the hook
                        # at ~50% -- the next layer's AG0 starts much earlier.
                        emit_edges(0, l, 0, 12)
                        build_l0_chunk(1)
                        emit_edges(0, l, 12, C0G)
                        emit_edges(1, l, 0, C0G, after_group=cb0)
                    else:
                        # full epoch-0 sweep first (its table is ready), then
                        # the first ep1 half; this keeps ep0 work ahead of any
                        # ep1 stall on the second table chunk's AllGather.
                        emit_edges(0, l, 0, NGRP)
                        emit_edges(1, l, 0, C0G, after_group=cb0)
                # h3 chunk-0 final: fire next layer's chunk-0 AllGather (the
                # per-group callback already built and stored the rows)
                if not last:
                    if parts < 2:
                        emit_leaky(l, 0)
                        build_chunk(l + 1, 0, hsrc_next)
                    emit_ag(l + 1, 0)
                elif cb0 is None:
                    emit_leaky(l, 0)
                if parts >= 2:
                    if l == 0:
                        emit_edges(0, l, C0G, NGRP)
                    emit_edges(1, l, C0G, NGRP, after_group=cb1)
                if parts < 3:
                    break
                if not last:
                    emit_ag(l + 1, 1)
                    emit_spart(l + 1, hsrc_next)
                elif cb1 is None:
                    emit_leaky(l, 1)
            if parts < 4:
                z_dbg = tail.tile([G, NCLS], F32, tag="zsb")
                nc.vector.tensor_copy(z_dbg[:], hs_pp[0][0:G, 0:NCLS])
                nc.sync.dma_start(t_out[:], z_dbg[:])
            else:
                pool_sb = tail.tile([G, H], F32, tag="poolsb")
                nc.vector.tensor_copy(pool_sb[:], pp[:])
                nc.sync.dma_start(pool_in[:], pool_sb[:])
                nc.gpsimd.collective_compute(
                    "AllGather", OP.bypass, replica_groups=[list(range(R))],
                    ins=[pool_in[:]], outs=[pool_out[:]])
                pr = tail.tile([G, R, H], F32, tag="pr")
                nc.sync.dma_start(pr[:], pool_out[:].rearrange("(r g) h -> g r h", r=R))
                pooled = tail.tile([G, H], F32, tag="pooled")
                nc.vector.tensor_tensor(out=pooled[:], in0=pr[:, 0, :], in1=pr[:, 1, :],
                                        op=OP.add)
                for r in range(2, R):
                    nc.vector.tensor_tensor(out=pooled[:], in0=pooled[:],
                                            in1=pr[:, r, :], op=OP.add)
                ptp = pedge.tile([H, G], F32, tag="pa")
                nc.tensor.transpose(out=ptp[:], in_=pooled[:], identity=identf[0:G, 0:G])
                pooledT = tail.tile([H, G], F32, tag="pooledT")
                nc.vector.tensor_copy(pooledT[:], ptp[:])
                zp = pedge.tile([G, NCLS], F32, tag="pa")
                nc.tensor.matmul(out=zp[:], lhsT=pooledT[:], rhs=Wch_t[:],
                                 start=True, stop=False)
                nc.tensor.matmul(out=zp[:], lhsT=clin_t[:], rhs=Wcc_t[:],
                                 start=False, stop=True)
                z_sb = tail.tile([G, NCLS], F32, tag="zsb")
                nc.vector.tensor_tensor(out=z_sb[:], in0=zp[:], in1=bc_t[:],
                                        op=OP.add)
                nc.sync.dma_start(t_out[:], z_sb[:])

    nc.compile()
    return nc


# ---------------------------------------------------------------------------

_CACHE = {}


def kernel(**inputs):
    in_maps, meta = prep(inputs)
    key = tuple(sorted((k, v) for k, v in meta.items()))
    if key not in _CACHE:
        _CACHE[key] = build(meta)
    nc = _CACHE[key]
    res = run_bass_kernel_spmd(nc, in_maps, list(range(R)))
    return np.asarray(res.results[0]["out"], np.float32)


def kernel_profiled(**inputs):
    """Like kernel() but also returns (exec_time_ns, trace_path)."""
    in_maps, meta = prep(inputs)
    key = tuple(sorted((k, v) for k, v in meta.items()))
    if key not in _CACHE:
        _CACHE[key] = build(meta)
    nc = _CACHE[key]
    res = run_bass_kernel_spmd(nc, in_maps, list(range(R)), trace=True)
    out = np.asarray(res.results[0]["out"], np.float32)
    trace_path = None
    if res.instructions_and_trace is not None:
        trace_path = res.instructions_and_trace[1]
    return out, res.exec_time_ns, trace_path


if __name__ == "__main__":
    pass


# revision 61
# speedup vs baseline: 1.0020x; 1.0020x over previous
"""Trainium2 Bass kernel for a 3-layer ResGatedGraphConv GNN (ClinicalGatedGCN).

Strategy (8 NeuronCores, SPMD), v3:
  - Nodes partitioned into 8 contiguous ranges of 6250 (padded to 6272 = 49
    groups of 128). Edges assigned to the rank owning their dst node, sorted
    by (src-chunk epoch, dst-group, dst) on the host.
  - The [q|v] node table is built DISTRIBUTED: each rank computes q,v (and k)
    only for its OWN nodes from its SBUF-resident h (one fused matmul per
    128-node group against [Wq|Wv|Wk]), stores the [q|v] rows to local DRAM,
    and AllGathers the table in two chunks (groups 0:24 -> qv_t0, 24:49 ->
    qv_t1; both tables stay under the int16 gather-index limit). This removes
    the per-layer h AllGather, all h re-reads, and ~7/8 of the table matmuls
    and PSUM->SBUF copies of the replicated design.
  - One dma_gather per (epoch, dst-group) fetches the src rows of [q|v].
    k[dst] is expanded on the PE via a host-shipped 0/1 selector ST; the
    segment-sum over dst is a PE matmul against selector S. S and ST are
    packed in ONE fp8 blob per (epoch, group) (0/1 is exact in fp8; matmul
    operands may mix fp8 with bf16), halving selector DMA traffic.
  - The edge-embedding rank-1 term (attr x We + gate bias) is batched: one
    K=5 matmul per 4-tile PSUM chunk against a block-diagonal [5, 4H]
    constant, with per-chunk attr columns shipped once as an fp8 constant.
  - BatchNorm folding: A = gamma/sqrt(var+eps) is folded into the NEXT
    layer's weights (and the classifier); B is folded into effective biases.
    The on-device h update is a single fused leaky-ReLU
    (scalar_tensor_tensor max(x, slope*x)).
  - Mean-pool per graph is a matmul against a host-built indicator with
    1/cnt folded in; partial pools are AllGather'd and summed; the tiny
    classifier runs on every core.
"""

import numpy as np
import ml_dtypes

import concourse.bacc as bacc
import concourse.bass as bass
import concourse.mybir as mybir
import concourse.tile as tile
from concourse.bass_utils import run_bass_kernel_spmd
from concourse.masks import make_identity

F32 = mybir.dt.float32
BF16 = mybir.dt.bfloat16
FP8 = mybir.dt.float8e4
I16 = mybir.dt.int16
AF = mybir.ActivationFunctionType
OP = mybir.AluOpType

# ---------------- problem constants (hardcoded per spec) ----------------
N, E, H, G, NCLIN, NCLS = 50000, 800000, 128, 64, 16, 2
NLAYER = 3
EPS = 1e-5
SLOPE = 0.01
R = 8                      # ranks / NeuronCores

NPR = (N + R - 1) // R     # real nodes per rank (6250)
NGRP = (NPR + 127) // 128  # 128-node groups per rank (49)
NPAD = NGRP * 128          # padded nodes per rank (6272)
C0G = 24                   # groups in AG chunk 0
C1G = NGRP - C0G           # groups in AG chunk 1 (25)
C0N = C0G * 128            # 3072 rows
C1N = C1G * 128            # 3200 rows
KQC = 4                    # psum chunk size in edge tiles (1 PSUM bank)


def wrap_idxs_block(idx):
    """Wrap one gather call's indices: idx j -> [j%16, j//16], tiled to 128 parts."""
    n = len(idx)
    assert n % 16 == 0
    w = np.asarray(idx, np.int16).reshape(n // 16, 16).T
    return np.tile(w, (8, 1))


# ---------------------------------------------------------------------------
# host-side preprocessing
# ---------------------------------------------------------------------------

def prep(inputs):
    x = np.asarray(inputs["x"], np.float32)
    edge_index = np.asarray(inputs["edge_index"])
    edge_attr = np.asarray(inputs["edge_attr"], np.float32)[:, 0]
    batch = np.asarray(inputs["batch"]).astype(np.int64)
    clinical = np.asarray(inputs["clinical"], np.float32)
    Wk, bk = np.asarray(inputs["Wk"], np.float32), np.asarray(inputs["bk"], np.float32)
    Wq, bq = np.asarray(inputs["Wq"], np.float32), np.asarray(inputs["bq"], np.float32)
    Wv, bv = np.asarray(inputs["Wv"], np.float32), np.asarray(inputs["bv"], np.float32)
    Ws, bs = np.asarray(inputs["Ws"], np.float32), np.asarray(inputs["bs"], np.float32)
    We, be = np.asarray(inputs["We"], np.float32), np.asarray(inputs["be"], np.float32)
    gamma = np.asarray(inputs["gamma"], np.float32)
    beta = np.asarray(inputs["beta"], np.float32)
    rmean = np.asarray(inputs["rmean"], np.float32)
    rvar = np.asarray(inputs["rvar"], np.float32)
    Wc, bc = np.asarray(inputs["Wc"], np.float32), np.asarray(inputs["bc"], np.float32)

    src = edge_index[0].astype(np.int64)
    dst = edge_index[1].astype(np.int64)

    # BN folded: true h_out = A*leaky(pre) + B; device h3 = leaky(pre).
    A = gamma / np.sqrt(rvar + EPS)          # [3, H]
    B = beta - rmean * A                     # [3, H]

    # effective weights: fold diag(A[l-1]) into layer-l input maps, and the
    # B[l-1] offset into layer-l biases.
    Wq_e = np.stack([Wq[l] * (A[l - 1][:, None] if l else 1.0) for l in range(NLAYER)])
    Wk_e = np.stack([Wk[l] * (A[l - 1][:, None] if l else 1.0) for l in range(NLAYER)])
    Wv_e = np.stack([Wv[l] * (A[l - 1][:, None] if l else 1.0) for l in range(NLAYER)])
    Ws_e = np.stack([Ws[l] * (A[l - 1][:, None] if l else 1.0) for l in range(NLAYER)])
    bgate = np.stack([bk[l] + bq[l] + be[l]
                      + (B[l - 1] @ (Wk[l] + Wq[l]) if l else 0.0)
                      for l in range(NLAYER)])
    bv_e = np.stack([bv[l] + (B[l - 1] @ Wv[l] if l else 0.0) for l in range(NLAYER)])
    bs_e = np.stack([bs[l] + (B[l - 1] @ Ws[l] if l else 0.0) for l in range(NLAYER)])
    Wc_h = Wc[0:H] * A[2][:, None]
    bc_e = bc + B[2] @ Wc[0:H]

    Wqvk = np.concatenate([Wq_e, Wv_e, Wk_e], axis=2)     # [3, H, 3H]

    # block-diagonal edge-term constant: rows 0..3 carry We on diag block,
    # row 4 carries the gate bias (broadcast to each tile block).
    BD = np.zeros((NLAYER, 5, KQC * H), np.float32)
    for l in range(NLAYER):
        for t in range(KQC):
            BD[l, t, t * H:(t + 1) * H] = We[l, 0, :]
            BD[l, 4, t * H:(t + 1) * H] = bgate[l]

    has_bv = bool(np.any(bv_e != 0))
    has_bs = bool(np.any(bs_e != 0))

    # ---- edge structure ----
    e_rank = dst // NPR
    dst_local = dst - e_rank * NPR
    grp = dst_local // 128
    drel = dst_local % 128
    rs = src // NPR
    lr = src - rs * NPR
    ep = (lr >= C0N).astype(np.int64)
    trow = np.where(ep == 0, rs * C0N + lr, rs * C1N + (lr - C0N))

    counts = np.zeros((2, R, NGRP), np.int64)
    np.add.at(counts, (ep, e_rank, grp), 1)
    nt_l = [np.ceil(counts[e].max(axis=0) / 128).astype(int) for e in (0, 1)]
    off_l = [np.concatenate([[0], np.cumsum(nt)]).astype(int) for nt in nt_l]
    nch_l = [np.ceil(nt / KQC).astype(int) for nt in nt_l]
    choff_l = [np.concatenate([[0], np.cumsum(nc)]).astype(int) for nc in nch_l]

    cntg = np.bincount(batch, minlength=G).astype(np.float32)
    inv_cnt = 1.0 / np.maximum(cntg, 1.0)

    order = np.lexsort((dst, grp, ep, e_rank))
    trow_s, drel_s, attr_s = trow[order], drel[order], edge_attr[order]
    key = (e_rank[order] * 2 + ep[order]) * NGRP + grp[order]
    starts = np.searchsorted(key, np.arange(R * 2 * NGRP + 1))

    in_maps = []
    for r in range(R):
        ep_arrs = {}
        for e in (0, 1):
            nt = nt_l[e]
            off = off_l[e]
            nch = nch_l[e]
            choff = choff_l[e]
            tot = int(off[-1])
            chtot = int(choff[-1])
            gidx = np.zeros((128, tot * 8), np.int16)
            blob = np.zeros((128, tot * 256), ml_dtypes.float8_e4m3)
            attr4 = np.zeros((5, chtot * 128), ml_dtypes.float8_e4m3)
            attr4[4, :] = 1.0
            for g in range(NGRP):
                ntg = int(nt[g])
                if ntg == 0:
                    continue
                k = (r * 2 + e) * NGRP + g
                s0 = int(starts[k])
                n = int(counts[e, r, g])
                o = int(off[g])
                co = int(choff[g])
                idx = np.full((ntg * 128,), -1, np.int64)
                idx[:n] = trow_s[s0:s0 + n]
                if n == 0:
                    idx[0] = 0
                gidx[:, o * 8:(o + ntg) * 8] = wrap_idxs_block(idx)
                j = np.arange(n)
                t = j // 128
                p = j % 128
                dr = drel_s[s0:s0 + n].astype(np.int64)
                # S: [p_edge, d] at cols (o+t)*256 + d
                blob[p, (o + t) * 256 + dr] = 1
                # ST: [drel, p_edge] at cols (o+t)*256 + 128 + p
                blob[dr, (o + t) * 256 + 128 + p] = 1
                attr4[t % KQC, (co + t // KQC) * 128 + p] = \
                    attr_s[s0:s0 + n].astype(ml_dtypes.float8_e4m3)
            ep_arrs[e] = (gidx, blob, attr4)
        IndT = np.zeros((NPAD, G), np.float32)
        lo, hi = r * NPR, min((r + 1) * NPR, N)
        IndT[np.arange(hi - lo), batch[lo:hi]] = inv_cnt[batch[lo:hi]]
        xT_loc = np.zeros((128, NPAD), np.float32)
        xT_loc[:, 0:hi - lo] = x[lo:hi].T
        x_rb = np.zeros((R * 128, NPAD), np.float32)
        for rr in range(R):
            rlo, rhi = rr * NPR, min((rr + 1) * NPR, N)
            x_rb[rr * 128:(rr + 1) * 128, 0:rhi - rlo] = x[rlo:rhi].T
        gcnt = np.maximum(counts[:, r, :], 1).astype(np.int32).reshape(1, 2 * NGRP)
        im = {
            "gcnt": gcnt,
            "xT_loc": xT_loc.astype(ml_dtypes.float8_e4m3),
            "x_rb": x_rb.astype(ml_dtypes.float8_e4m3),
            "Wqvk": Wqvk.astype(ml_dtypes.bfloat16),
            "Ws": Ws_e.astype(ml_dtypes.bfloat16),
            "BD": BD.astype(ml_dtypes.bfloat16),
            "bv_row": np.concatenate(
                [np.zeros((NLAYER, 1, H), np.float32),
                 bv_e.reshape(NLAYER, 1, H),
                 np.zeros((NLAYER, 1, H), np.float32)], axis=2
            ).astype(ml_dtypes.bfloat16),
            "bs_col": bs_e.reshape(NLAYER, H, 1),
            "gidx0": ep_arrs[0][0], "blob0": ep_arrs[0][1], "attr0": ep_arrs[0][2],
            "gidx1": ep_arrs[1][0], "blob1": ep_arrs[1][1], "attr1": ep_arrs[1][2],
            "IndT": IndT.astype(ml_dtypes.bfloat16),
            "clinT": clinical.T.copy(),
            "Wc_h": Wc_h, "Wc_c": Wc[H:H + NCLIN],
            "bc_rep": np.tile(bc_e, (G, 1)),
        }
        in_maps.append(im)
    meta = dict(NT0=tuple(int(v) for v in nt_l[0]),
                NT1=tuple(int(v) for v in nt_l[1]),
                has_bv=has_bv, has_bs=has_bs, tab_fp8=True, dr_seg=True)
    return in_maps, meta


# ---------------------------------------------------------------------------
# device program
# ---------------------------------------------------------------------------

def build(meta):
    parts = meta.get("parts", 4)
    DT = BF16
    TDT = FP8 if meta.get("tab_fp8", True) else BF16
    NT_L = [list(meta["NT0"]), list(meta["NT1"])]
    OFF_L = [np.concatenate([[0], np.cumsum(nt)]).astype(int) for nt in NT_L]
    NCH_L = [np.ceil(np.array(nt) / KQC).astype(int) for nt in NT_L]
    CHOFF_L = [np.concatenate([[0], np.cumsum(nc)]).astype(int) for nc in NCH_L]
    TOT = [int(o[-1]) for o in OFF_L]
    CHTOT = [int(c[-1]) for c in CHOFF_L]
    TMAX = max(max(NT_L[0]), max(NT_L[1]))

    nc = bacc.Bacc("TRN2", target_bir_lowering=False, debug=False, num_devices=R)

    def din(name, shape, dt):
        return nc.dram_tensor(name, shape, dt, kind="ExternalInput").ap()

    t_xT = din("xT_loc", [128, NPAD], FP8)
    t_cnt = din("gcnt", [1, 2 * NGRP], mybir.dt.int32)
    t_xrb = din("x_rb", [R * 128, NPAD], FP8)
    t_Wqvk = din("Wqvk", [NLAYER, H, 3 * H], DT)
    t_Ws = din("Ws", [NLAYER, H, H], DT)
    t_BD = din("BD", [NLAYER, 5, KQC * H], DT)
    t_bv = din("bv_row", [NLAYER, 1, 3 * H], DT)
    t_bs = din("bs_col", [NLAYER, H, 1], F32)
    t_gidx = [din("gidx0", [128, TOT[0] * 8], I16),
              din("gidx1", [128, TOT[1] * 8], I16)]
    t_blob = [din("blob0", [128, TOT[0] * 256], FP8),
              din("blob1", [128, TOT[1] * 256], FP8)]
    t_attr = [din("attr0", [5, CHTOT[0] * 128], FP8),
              din("attr1", [5, CHTOT[1] * 128], FP8)]
    t_IndT = din("IndT", [NPAD, G], DT)
    t_clinT = din("clinT", [NCLIN, G], F32)
    t_Wc_h = din("Wc_h", [H, NCLS], F32)
    t_Wc_c = din("Wc_c", [NCLIN, NCLS], F32)
    t_bc = din("bc_rep", [G, NCLS], F32)

    t_out = nc.dram_tensor("out", [G, NCLS], F32, kind="ExternalOutput").ap()

    qv_loc = nc.dram_tensor("qv_loc", [NPAD, 2 * H], TDT).ap()
    # per-layer AG'd table pairs (separate per layer so the next layer's
    # AllGather never overwrites a table the current layer still gathers from)
    qv_tl = [None,
             [nc.dram_tensor("qv1_t0", [R * C0N, 2 * H], TDT, addr_space="Shared").ap(),
              nc.dram_tensor("qv1_t1", [R * C1N, 2 * H], TDT, addr_space="Shared").ap()],
             [nc.dram_tensor("qv2_t0", [R * C0N, 2 * H], TDT, addr_space="Shared").ap(),
              nc.dram_tensor("qv2_t1", [R * C1N, 2 * H], TDT, addr_space="Shared").ap()]]
    # layer-0 tables are built locally (x is replicated), no collective
    qv_tl[0] = [nc.dram_tensor("qv0_t0", [R * C0N, 2 * H], TDT).ap(),
                nc.dram_tensor("qv0_t1", [R * C1N, 2 * H], TDT).ap()]
    pool_in = nc.dram_tensor("pool_in", [G, H], F32).ap()
    pool_out = nc.dram_tensor("pool_out", [R * G, H], F32, addr_space="Shared").ap()

    with tile.TileContext(nc) as tc:
        import contextlib
        with contextlib.ExitStack() as ctx:
            consts = ctx.enter_context(tc.tile_pool(name="consts", bufs=1))
            hsb = ctx.enter_context(tc.tile_pool(name="hsb", bufs=1))
            h3p = ctx.enter_context(tc.tile_pool(name="h3p", bufs=1))
            ksb = ctx.enter_context(tc.tile_pool(name="ksb", bufs=1))
            stg = ctx.enter_context(tc.tile_pool(name="stg", bufs=4))
            tail = ctx.enter_context(tc.tile_pool(name="tail", bufs=2))
            edg = ctx.enter_context(tc.tile_pool(name="edg", bufs=5))
            sel = ctx.enter_context(tc.tile_pool(name="sel", bufs=8))
            edm = ctx.enter_context(tc.tile_pool(name="edm", bufs=3))
            pnode = ctx.enter_context(tc.tile_pool(name="pnode", bufs=3, space="PSUM"))
            pkq = ctx.enter_context(tc.tile_pool(name="pkq", bufs=2, space="PSUM"))
            pedge = ctx.enter_context(tc.tile_pool(name="pedge", bufs=2, space="PSUM"))
            ppool = ctx.enter_context(tc.tile_pool(name="ppool", bufs=1, space="PSUM"))

            _cid = [0]

            def load_const(src_ap, shape, dt):
                _cid[0] += 1
                t = consts.tile(shape, dt, tag=f"c{_cid[0]}_{src_ap.tensor.name}")
                nc.sync.dma_start(t[:], src_ap)
                return t

            Wqvk_t = [load_const(t_Wqvk[l], [H, 3 * H], DT) for l in range(NLAYER)]
            Ws_t = [load_const(t_Ws[l], [H, H], DT) for l in range(NLAYER)]
            BD_t = [load_const(t_BD[l], [5, KQC * H], DT) for l in range(NLAYER)]
            bv_t = ([load_const(t_bv[l], [1, 3 * H], DT) for l in range(NLAYER)]
                    if meta["has_bv"] else None)
            bs_t = ([load_const(t_bs[l], [H, 1], F32) for l in range(NLAYER)]
                    if meta["has_bs"] else None)
            ones_t = consts.tile([1, 128], DT)
            nc.vector.memset(ones_t[:], 1.0)
            gidx_t = [load_const(t_gidx[0], [128, TOT[0] * 8], I16),
                      load_const(t_gidx[1], [128, TOT[1] * 8], I16)]
            attr_t = [load_const(t_attr[0], [5, CHTOT[0] * 128], FP8),
                      load_const(t_attr[1], [5, CHTOT[1] * 128], FP8)]
            xT_t = load_const(t_xT, [128, NPAD], FP8)
            cnt_t = load_const(t_cnt, [1, 2 * NGRP], mybir.dt.int32)
            ident = consts.tile([128, 128], DT)
            make_identity(nc, ident[:])
            identf = consts.tile([128, 128], F32)
            make_identity(nc, identf[:])
            clin_t = load_const(t_clinT, [NCLIN, G], F32)
            Wch_t = load_const(t_Wc_h, [H, NCLS], F32)
            Wcc_t = load_const(t_Wc_c, [NCLIN, NCLS], F32)
            bc_t = load_const(t_bc, [G, NCLS], F32)

            # ping-pong state by layer parity
            hs_pp = [hsb.tile([128, NPAD], DT, tag="hs0", name="hs0"),
                     hsb.tile([128, NPAD], DT, tag="hs1", name="hs1")]
            k_pp = [ksb.tile([128, NGRP, H], DT, tag="k0", name="k0"),
                    ksb.tile([128, NGRP, H], DT, tag="k1", name="k1")]
            h3_pp = [h3p.tile([128, NPAD], DT, tag="h3a", name="h3a"),
                     h3p.tile([128, NPAD], DT, tag="h3b", name="h3b")]
            h3f = h3_pp[0]                             # layer-2 output parity

            def build_chunk(l, c, hsrc):
                """Local q|v|k for groups of AG-chunk c of layer l."""
                g0, g1 = (0, C0G) if c == 0 else (C0G, NGRP)
                k_sb = k_pp[l % 2]
                g = g0
                while g < g1:
                    ns = min(8, g1 - g)
                    st = stg.tile([128, 8, 2 * H], TDT, tag="st")
                    for s in range(ns):
                        gg = g + s
                        ps = pnode.tile([128, 512], F32, tag="pn")
                        nc.tensor.matmul(
                            out=ps[:, 0:3 * H],
                            lhsT=hsrc[:, gg * 128:(gg + 1) * 128],
                            rhs=Wqvk_t[l][:], start=True,
                            stop=not meta["has_bv"],
                            skip_group_check=True)
                        if meta["has_bv"]:
                            nc.tensor.matmul(
                                out=ps[:, 0:3 * H], lhsT=ones_t[:],
                                rhs=bv_t[l][:], start=False, stop=True,
                                skip_group_check=True)
                        nc.scalar.activation(st[:, s, :], ps[:, 0:2 * H],
                                             AF.Copy)
                        nc.scalar.activation(k_sb[:, gg, :],
                                             ps[:, 2 * H:3 * H], AF.Copy)
                    nc.sync.dma_start(
                        qv_loc[g * 128:(g + ns) * 128, :].rearrange(
                            "(s p) f -> p s f", p=128),
                        st[:, 0:ns, :])
                    g += ns

            def emit_ag(l, c):
                ins = qv_loc[0:C0N, :] if c == 0 else qv_loc[C0N:NPAD, :]
                nc.gpsimd.collective_compute(
                    "AllGather", OP.bypass, replica_groups=[list(range(R))],
                    ins=[ins], outs=[qv_tl[l][c][:]])

            def emit_spart(l, hsrc):
                hs = hs_pp[l % 2]
                for c0 in range(0, NPAD, 512):
                    csz = min(512, NPAD - c0)
                    pss = pnode.tile([128, 512], F32, tag="pn")
                    nc.tensor.matmul(out=pss[:, 0:csz], lhsT=Ws_t[l][:],
                                     rhs=hsrc[:, c0:c0 + csz],
                                     start=True, stop=True)
                    if meta["has_bs"]:
                        nc.scalar.activation(hs[:, c0:c0 + csz], pss[:, 0:csz],
                                             AF.Identity, bias=bs_t[l][:],
                                             scale=1.0)
                    else:
                        nc.scalar.activation(hs[:, c0:c0 + csz], pss[:, 0:csz],
                                             AF.Copy)

            def make_tail_cb(l, c_start, c_end):
                # During layer l's ep1 sweep over [c_start, c_end): as each
                # dst-group finalizes, leaky just that slice and build the
                # next layer's table rows for it, storing every 8 groups so
                # the chunk AllGather can fire the moment the sweep ends.
                st_state = {}

                def cb(g, l=l):
                    sl = slice(g * 128, (g + 1) * 128)
                    nc.vector.scalar_tensor_tensor(
                        out=h3_pp[l % 2][:, sl], in0=hs_pp[l % 2][:, sl],
                        scalar=float(SLOPE), in1=hs_pp[l % 2][:, sl],
                        op0=OP.mult, op1=OP.max)
                    if l >= NLAYER - 1:
                        return
                    s = (g - c_start) % 8
                    if s == 0:
                        st_state["st"] = stg.tile([128, 8, 2 * H], TDT,
                                                  tag="st", name="st_cb")
                        st_state["g0"] = g
                    st = st_state["st"]
                    ps = pnode.tile([128, 512], F32, tag="pn", name="pn_cb")
                    nc.tensor.matmul(
                        out=ps[:, 0:3 * H],
                        lhsT=h3_pp[l % 2][:, sl],
                        rhs=Wqvk_t[l + 1][:], start=True,
                        stop=not meta["has_bv"], skip_group_check=True)
                    if meta["has_bv"]:
                        nc.tensor.matmul(
                            out=ps[:, 0:3 * H], lhsT=ones_t[:],
                            rhs=bv_t[l + 1][:], start=False, stop=True,
                            skip_group_check=True)
                    nc.scalar.activation(st[:, s, :], ps[:, 0:2 * H], AF.Copy)
                    nc.scalar.activation(k_pp[(l + 1) % 2][:, g, :],
                                         ps[:, 2 * H:3 * H], AF.Copy)
                    if s == 7 or g == c_end - 1:
                        g0b = st_state["g0"]
                        nc.sync.dma_start(
                            qv_loc[g0b * 128:(g + 1) * 128, :].rearrange(
                                "(s p) f -> p s f", p=128),
                            st[:, 0:g + 1 - g0b, :])
                return cb

            def emit_leaky(l, c):
                sl = slice(0, C0N) if c == 0 else slice(C0N, NPAD)
                nc.vector.scalar_tensor_tensor(
                    out=h3_pp[l % 2][:, sl], in0=hs_pp[l % 2][:, sl],
                    scalar=float(SLOPE), in1=hs_pp[l % 2][:, sl],
                    op0=OP.mult, op1=OP.max)

            def emit_edges(e, l, g0, g1, after_group=None):
                nt_list = NT_L[e]
                off = OFF_L[e]
                choff = CHOFF_L[e]
                hs = hs_pp[l % 2]
                k_sb = k_pp[l % 2]
                tab = qv_tl[l][e]
                for g in range(g0, g1):
                    nt = int(nt_list[g])
                    if nt == 0:
                        if after_group is not None:
                            after_group(g)
                        continue
                    ne = nt * 128
                    o = int(off[g])
                    co = int(choff[g])
                    bt = sel.tile([128, TMAX, 256], FP8, tag="blob")
                    nc.sync.dma_start(
                        bt[:, 0:nt, :],
                        t_blob[e][:, o * 256:(o + nt) * 256].rearrange(
                            "p (t d) -> p t d", t=nt))
                    gt = edg.tile([128, TMAX, 2 * H], TDT, tag="g")
                    nreg = nc.gpsimd.value_load(
                        cnt_t[0:1, e * NGRP + g:e * NGRP + g + 1])
                    nc.gpsimd.dma_gather(
                        gt[:, 0:nt, :], tab[:],
                        gidx_t[e][:, o * 8:(o + nt) * 8],
                        ne, nreg, 2 * H, single_packet=(ne <= 512))
                    ktb = edm.tile([128, TMAX, 128], DT, tag="kt")
                    for ci in range((nt + KQC - 1) // KQC):
                        q0 = ci * KQC
                        qn = min(KQC, nt - q0)
                        pk = pkq.tile([128, KQC, 128], F32, tag="kq")
                        nc.tensor.matmul(
                            out=pk[:, 0:KQC, :],
                            lhsT=attr_t[e][0:5, (co + ci) * 128:(co + ci + 1) * 128],
                            rhs=BD_t[l][:], start=True, stop=False,
                            skip_group_check=True)
                        for t in range(q0, q0 + qn):
                            nc.tensor.matmul(
                                out=pk[:, t - q0, :],
                                lhsT=bt[:, t, 128:256],
                                rhs=k_sb[:, g, :], start=False, stop=False,
                                skip_group_check=True)
                        # q[src] folded into the same PSUM via identity matmul
                        nc.tensor.matmul(
                            out=pk[:, 0:qn, :], lhsT=ident[:],
                            rhs=gt[:, q0:q0 + qn, 0:H], start=False, stop=True,
                            skip_group_check=True)
                        nc.scalar.activation(ktb[:, q0:q0 + qn, :],
                                             pk[:, 0:qn, :], AF.Sigmoid)
                    dr = meta.get("dr_seg", False)
                    msg = edm.tile([128, TMAX, 128], FP8 if dr else DT, tag="msg")
                    nc.vector.tensor_tensor(out=msg[:, 0:nt, :],
                                            in0=ktb[:, 0:nt, :],
                                            in1=gt[:, 0:nt, H:2 * H], op=OP.mult)
                    pa = pedge.tile([128, 128], F32, tag="pa")
                    if dr:
                        npair = nt // 2
                        for t2 in range(0, npair * 2, 2):
                            nc.tensor.matmul(
                                out=pa[:], lhsT=msg[:, t2:t2 + 2, :],
                                rhs=bt[:, t2:t2 + 2, 0:128], start=(t2 == 0),
                                stop=(t2 + 2 == nt),
                                perf_mode=mybir.MatmulPerfMode.DoubleRow,
                                skip_group_check=True)
                        if nt % 2:
                            nc.tensor.matmul(out=pa[:], lhsT=msg[:, nt - 1, :],
                                             rhs=bt[:, nt - 1, 0:128],
                                             start=(nt == 1), stop=True,
                                             skip_group_check=True)
                    else:
                        for t in range(nt):
                            nc.tensor.matmul(out=pa[:], lhsT=msg[:, t, :],
                                             rhs=bt[:, t, 0:128], start=(t == 0),
                                             stop=(t == nt - 1))
                    nc.vector.tensor_tensor(
                        out=hs[:, g * 128:(g + 1) * 128],
                        in0=hs[:, g * 128:(g + 1) * 128], in1=pa[:], op=OP.add)
                    if after_group is not None:
                        after_group(g)

            # ---- layer-0 table from x: replicated input, so each rank builds
            # the FULL table locally (no collective, no exposed startup AG).
            def build_l0_chunk(c):
                g0, g1, tab, cbase = ((0, C0G, qv_tl[0][0], C0N) if c == 0
                                      else (C0G, NGRP, qv_tl[0][1], C1N))
                for rb in range(R):
                    g = g0
                    while g < g1:
                        ns = min(8, g1 - g)
                        lh = stg.tile([128, 1024], FP8, tag="lh")
                        nc.sync.dma_start(
                            lh[:, 0:ns * 128],
                            t_xrb[rb * 128:(rb + 1) * 128,
                                  g * 128:(g + ns) * 128])
                        st = stg.tile([128, 8, 2 * H], TDT, tag="st")
                        for s in range(0, ns, 2):
                            n2 = min(2, ns - s)
                            ps = pnode.tile([128, 512], F32, tag="pn")
                            for u in range(n2):
                                nc.tensor.matmul(
                                    out=ps[:, u * 256:u * 256 + 2 * H],
                                    lhsT=lh[:, (s + u) * 128:(s + u + 1) * 128],
                                    rhs=Wqvk_t[0][:, 0:2 * H], start=True,
                                    stop=not meta["has_bv"],
                                    skip_group_check=True)
                                if meta["has_bv"]:
                                    nc.tensor.matmul(
                                        out=ps[:, u * 256:u * 256 + 2 * H],
                                        lhsT=ones_t[:],
                                        rhs=bv_t[0][0:1, 0:2 * H],
                                        start=False, stop=True,
                                        skip_group_check=True)
                            if (s // 2) % 2 == 0:
                                nc.scalar.activation(st[:, s:s + n2, :],
                                                     ps[:, 0:n2 * 256], AF.Copy)
                            else:
                                nc.vector.tensor_copy(st[:, s:s + n2, :],
                                                      ps[:, 0:n2 * 256])
                        r0 = rb * cbase + (g - g0) * 128
                        nc.sync.dma_start(
                            tab[r0:r0 + ns * 128, :].rearrange(
                                "(s p) f -> p s f", p=128),
                            st[:, 0:ns, :])
                        g += ns
            build_l0_chunk(0)
            # local k table + s-part for layer 0 from the local x slice
            for g in range(NGRP):
                psk = pnode.tile([128, 512], F32, tag="pn")
                nc.tensor.matmul(out=psk[:, 0:H],
                                 lhsT=xT_t[:, g * 128:(g + 1) * 128],
                                 rhs=Wqvk_t[0][:, 2 * H:3 * H],
                                 start=True, stop=True, skip_group_check=True)
                nc.scalar.activation(k_pp[0][:, g, :], psk[:, 0:H], AF.Copy)
            emit_spart(0, xT_t)

            for _ in range(5):
                gz = edg.tile([128, TMAX, 2 * H], TDT, tag="g")
                nc.vector.memset(gz[:], 0.0)

            pp = ppool.tile([G, H], F32)
            indc = consts.tile([128, NGRP, G], DT)
            nc.sync.dma_start(
                indc[:], t_IndT[:].rearrange("(c p) g -> p c g", p=128))

            def make_pool_cb(l):
                def cb(g, l=l):
                    sl = slice(g * 128, (g + 1) * 128)
                    nc.vector.scalar_tensor_tensor(
                        out=h3_pp[l % 2][:, sl], in0=hs_pp[l % 2][:, sl],
                        scalar=float(SLOPE), in1=hs_pp[l % 2][:, sl],
                        op0=OP.mult, op1=OP.max)
                    trp = pedge.tile([128, 128], DT, tag="pa", name="trp_cb")
                    nc.tensor.transpose(out=trp[:], in_=h3f[:, sl],
                                        identity=ident[:])
                    hnode = tail.tile([128, 128], DT, tag="hnode",
                                      name="hnode_cb")
                    nc.vector.tensor_copy(hnode[:], trp[:])
                    nc.tensor.matmul(out=pp[:], lhsT=indc[:, g, :],
                                     rhs=hnode[:],
                                     start=(g == 0), stop=(g == NGRP - 1))
                return cb

            def emit_pool_part(c0g, c1g):
                for c in range(c0g, c1g):
                    trp = pedge.tile([128, 128], DT, tag="pa")
                    nc.tensor.transpose(out=trp[:],
                                        in_=h3f[:, c * 128:(c + 1) * 128],
                                        identity=ident[:])
                    hnode = tail.tile([128, 128], DT, tag="hnode")
                    nc.vector.tensor_copy(hnode[:], trp[:])
                    nc.tensor.matmul(out=pp[:],
                                     lhsT=indc[:, c, :],
                                     rhs=hnode[:],
                                     start=(c == 0), stop=(c == NGRP - 1))

            for l in range(NLAYER):
                hsrc_next = h3_pp[l % 2]
                last = l >= NLAYER - 1
                if parts >= 3 and not last:
                    cb0 = make_tail_cb(l, 0, C0G)
                    cb1 = make_tail_cb(l, C0G, NGRP)
                elif parts >= 4 and last:
                    cb0 = cb1 = make_pool_cb(l)
                else:
                    cb0 = cb1 = None
                if parts >= 2:
                    if l == 0:
                        # layer 0 consumes no AllGather (local tables), so run
                        # both epochs' first halves up front and fire the hook
                        # at ~50% -- the next layer's AG0 starts much earlier.
                        emit_edges(0, l, 0, 12)
                        build_l0_chunk(1)
                        emit_edges(0, l, 12, C0G)
                        emit_edges(1, l, 0, C0G, after_group=cb0)
                    else:
                        # full epoch-0 sweep first (its table is ready), then
                        # the first ep1 half; this keeps ep0 work ahead of any
                        # ep1 stall on the second table chunk's AllGather.
                        emit_edges(0, l, 0, NGRP)
                        emit_edges(1, l, 0, C0G, after_group=cb0)
                # h3 chunk-0 final: fire next layer's chunk-0 AllGather (the
                # per-group callback already built and stored the rows)
                if not last:
                    if parts < 2:
                        emit_leaky(l, 0)
                        build_chunk(l + 1, 0, hsrc_next)
                    emit_ag(l + 1, 0)
                elif cb0 is None:
                    emit_leaky(l, 0)
                if parts >= 2:
                    if l == 0:
                        emit_edges(0, l, C0G, NGRP)
                    emit_edges(1, l, C0G, NGRP, after_group=cb1)
                if parts < 3:
                    break
                if not last:
                    emit_ag(l + 1, 1)
                    emit_spart(l + 1, hsrc_next)
                elif cb1 is None:
                    emit_leaky(l, 1)
            if parts < 4:
                z_dbg = tail.tile([G, NCLS], F32, tag="zsb")
                nc.vector.tensor_copy(z_dbg[:], hs_pp[0][0:G, 0:NCLS])
                nc.sync.dma_start(t_out[:], z_dbg[:])
            else:
                pool_sb = tail.tile([G, H], F32, tag="poolsb")
                nc.vector.tensor_copy(pool_sb[:], pp[:])
                nc.sync.dma_start(pool_in[:], pool_sb[:])
                nc.gpsimd.collective_compute(
                    "AllGather", OP.bypass, replica_groups=[list(range(R))],
                    ins=[pool_in[:]], outs=[pool_out[:]])
                pr = tail.tile([G, R, H], F32, tag="pr")
                nc.sync.dma_start(pr[:], pool_out[:].rearrange("(r g) h -> g r h", r=R))
                pooled = tail.tile([G, H], F32, tag="pooled")
                nc.vector.tensor_tensor(out=pooled[:], in0=pr[:, 0, :], in1=pr[:, 1, :],
                                        op=OP.add)
                for r in range(2, R):
                    nc.vector.tensor_tensor(out=pooled[:], in0=pooled[:],
                                            in1=pr[:, r, :], op=OP.add)
                ptp = pedge.tile([H, G], F32, tag="pa")
                nc.tensor.transpose(out=ptp[:], in_=pooled[:], identity=identf[0:G, 0:G])
                pooledT = tail.tile([H, G], F32, tag="pooledT")
                nc.vector.tensor_copy(pooledT[:], ptp[:])
                zp = pedge.tile([G, NCLS], F32, tag="pa")
                nc.tensor.matmul(out=zp[:], lhsT=pooledT[:], rhs=Wch_t[:],
                                 start=True, stop=False)
                nc.tensor.matmul(out=zp[:], lhsT=clin_t[:], rhs=Wcc_t[:],
                                 start=False, stop=True)
                z_sb = tail.tile([G, NCLS], F32, tag="zsb")
                nc.vector.tensor_tensor(out=z_sb[:], in0=zp[:], in1=bc_t[:],
                                        op=OP.add)
                nc.sync.dma_start(t_out[:], z_sb[:])

    nc.compile()
    return nc


# ---------------------------------------------------------------------------

_CACHE = {}


def kernel(**inputs):
    in_maps, meta = prep(inputs)
    key = tuple(sorted((k, v) for k, v in meta.items()))
    if key not in _CACHE:
        _CACHE[key] = build(meta)
    nc = _CACHE[key]
    res = run_bass_kernel_spmd(nc, in_maps, list(range(R)))
    return np.asarray(res.results[0]["out"], np.float32)


def kernel_profiled(**inputs):
    """Like kernel() but also returns (exec_time_ns, trace_path)."""
    in_maps, meta = prep(inputs)
    key = tuple(sorted((k, v) for k, v in meta.items()))
    if key not in _CACHE:
        _CACHE[key] = build(meta)
    nc = _CACHE[key]
    res = run_bass_kernel_spmd(nc, in_maps, list(range(R)), trace=True)
    out = np.asarray(res.results[0]["out"], np.float32)
    trace_path = None
    if res.instructions_and_trace is not None:
        trace_path = res.instructions_and_trace[1]
    return out, res.exec_time_ns, trace_path


if __name__ == "__main__":
    pass


# revision 66
# speedup vs baseline: 1.0085x; 1.0064x over previous
"""Trainium2 Bass kernel for a 3-layer ResGatedGraphConv GNN (ClinicalGatedGCN).

Strategy (8 NeuronCores, SPMD), v3:
  - Nodes partitioned into 8 contiguous ranges of 6250 (padded to 6272 = 49
    groups of 128). Edges assigned to the rank owning their dst node, sorted
    by (src-chunk epoch, dst-group, dst) on the host.
  - The [q|v] node table is built DISTRIBUTED: each rank computes q,v (and k)
    only for its OWN nodes from its SBUF-resident h (one fused matmul per
    128-node group against [Wq|Wv|Wk]), stores the [q|v] rows to local DRAM,
    and AllGathers the table in two chunks (groups 0:24 -> qv_t0, 24:49 ->
    qv_t1; both tables stay under the int16 gather-index limit). This removes
    the per-layer h AllGather, all h re-reads, and ~7/8 of the table matmuls
    and PSUM->SBUF copies of the replicated design.
  - One dma_gather per (epoch, dst-group) fetches the src rows of [q|v].
    k[dst] is expanded on the PE via a host-shipped 0/1 selector ST; the
    segment-sum over dst is a PE matmul against selector S. S and ST are
    packed in ONE fp8 blob per (epoch, group) (0/1 is exact in fp8; matmul
    operands may mix fp8 with bf16), halving selector DMA traffic.
  - The edge-embedding rank-1 term (attr x We + gate bias) is batched: one
    K=5 matmul per 4-tile PSUM chunk against a block-diagonal [5, 4H]
    constant, with per-chunk attr columns shipped once as an fp8 constant.
  - BatchNorm folding: A = gamma/sqrt(var+eps) is folded into the NEXT
    layer's weights (and the classifier); B is folded into effective biases.
    The on-device h update is a single fused leaky-ReLU
    (scalar_tensor_tensor max(x, slope*x)).
  - Mean-pool per graph is a matmul against a host-built indicator with
    1/cnt folded in; partial pools are AllGather'd and summed; the tiny
    classifier runs on every core.
"""

import numpy as np
import ml_dtypes

import concourse.bacc as bacc
import concourse.bass as bass
import concourse.mybir as mybir
import concourse.tile as tile
from concourse.bass_utils import run_bass_kernel_spmd
from concourse.masks import make_identity

F32 = mybir.dt.float32
BF16 = mybir.dt.bfloat16
FP8 = mybir.dt.float8e4
I16 = mybir.dt.int16
AF = mybir.ActivationFunctionType
OP = mybir.AluOpType

# ---------------- problem constants (hardcoded per spec) ----------------
N, E, H, G, NCLIN, NCLS = 50000, 800000, 128, 64, 16, 2
NLAYER = 3
EPS = 1e-5
SLOPE = 0.01
R = 8                      # ranks / NeuronCores

NPR = (N + R - 1) // R     # real nodes per rank (6250)
NGRP = (NPR + 127) // 128  # 128-node groups per rank (49)
NPAD = NGRP * 128          # padded nodes per rank (6272)
C0G = 24                   # groups in AG chunk 0
C1G = NGRP - C0G           # groups in AG chunk 1 (25)
C0N = C0G * 128            # 3072 rows
C1N = C1G * 128            # 3200 rows
KQC = 4                    # psum chunk size in edge tiles (1 PSUM bank)


def wrap_idxs_block(idx):
    """Wrap one gather call's indices: idx j -> [j%16, j//16], tiled to 128 parts."""
    n = len(idx)
    assert n % 16 == 0
    w = np.asarray(idx, np.int16).reshape(n // 16, 16).T
    return np.tile(w, (8, 1))


# ---------------------------------------------------------------------------
# host-side preprocessing
# ---------------------------------------------------------------------------

def prep(inputs):
    x = np.asarray(inputs["x"], np.float32)
    edge_index = np.asarray(inputs["edge_index"])
    edge_attr = np.asarray(inputs["edge_attr"], np.float32)[:, 0]
    batch = np.asarray(inputs["batch"]).astype(np.int64)
    clinical = np.asarray(inputs["clinical"], np.float32)
    Wk, bk = np.asarray(inputs["Wk"], np.float32), np.asarray(inputs["bk"], np.float32)
    Wq, bq = np.asarray(inputs["Wq"], np.float32), np.asarray(inputs["bq"], np.float32)
    Wv, bv = np.asarray(inputs["Wv"], np.float32), np.asarray(inputs["bv"], np.float32)
    Ws, bs = np.asarray(inputs["Ws"], np.float32), np.asarray(inputs["bs"], np.float32)
    We, be = np.asarray(inputs["We"], np.float32), np.asarray(inputs["be"], np.float32)
    gamma = np.asarray(inputs["gamma"], np.float32)
    beta = np.asarray(inputs["beta"], np.float32)
    rmean = np.asarray(inputs["rmean"], np.float32)
    rvar = np.asarray(inputs["rvar"], np.float32)
    Wc, bc = np.asarray(inputs["Wc"], np.float32), np.asarray(inputs["bc"], np.float32)

    src = edge_index[0].astype(np.int64)
    dst = edge_index[1].astype(np.int64)

    # BN folded: true h_out = A*leaky(pre) + B; device h3 = leaky(pre).
    A = gamma / np.sqrt(rvar + EPS)          # [3, H]
    B = beta - rmean * A                     # [3, H]

    # effective weights: fold diag(A[l-1]) into layer-l input maps, and the
    # B[l-1] offset into layer-l biases.
    Wq_e = np.stack([Wq[l] * (A[l - 1][:, None] if l else 1.0) for l in range(NLAYER)])
    Wk_e = np.stack([Wk[l] * (A[l - 1][:, None] if l else 1.0) for l in range(NLAYER)])
    Wv_e = np.stack([Wv[l] * (A[l - 1][:, None] if l else 1.0) for l in range(NLAYER)])
    Ws_e = np.stack([Ws[l] * (A[l - 1][:, None] if l else 1.0) for l in range(NLAYER)])
    bgate = np.stack([bk[l] + bq[l] + be[l]
                      + (B[l - 1] @ (Wk[l] + Wq[l]) if l else 0.0)
                      for l in range(NLAYER)])
    bv_e = np.stack([bv[l] + (B[l - 1] @ Wv[l] if l else 0.0) for l in range(NLAYER)])
    bs_e = np.stack([bs[l] + (B[l - 1] @ Ws[l] if l else 0.0) for l in range(NLAYER)])
    Wc_h = Wc[0:H] * A[2][:, None]
    bc_e = bc + B[2] @ Wc[0:H]

    Wqvk = np.concatenate([Wq_e, Wv_e, Wk_e], axis=2)     # [3, H, 3H]

    # block-diagonal edge-term constant: rows 0..3 carry We on diag block,
    # row 4 carries the gate bias (broadcast to each tile block).
    BD = np.zeros((NLAYER, 5, KQC * H), np.float32)
    for l in range(NLAYER):
        for t in range(KQC):
            BD[l, t, t * H:(t + 1) * H] = We[l, 0, :]
            BD[l, 4, t * H:(t + 1) * H] = bgate[l]

    has_bv = bool(np.any(bv_e != 0))
    has_bs = bool(np.any(bs_e != 0))

    # ---- edge structure ----
    e_rank = dst // NPR
    dst_local = dst - e_rank * NPR
    grp = dst_local // 128
    drel = dst_local % 128
    rs = src // NPR
    lr = src - rs * NPR
    ep = (lr >= C0N).astype(np.int64)
    trow = np.where(ep == 0, rs * C0N + lr, rs * C1N + (lr - C0N))

    counts = np.zeros((2, R, NGRP), np.int64)
    np.add.at(counts, (ep, e_rank, grp), 1)
    nt_l = [np.ceil(counts[e].max(axis=0) / 128).astype(int) for e in (0, 1)]
    off_l = [np.concatenate([[0], np.cumsum(nt)]).astype(int) for nt in nt_l]
    nch_l = [np.ceil(nt / KQC).astype(int) for nt in nt_l]
    choff_l = [np.concatenate([[0], np.cumsum(nc)]).astype(int) for nc in nch_l]

    cntg = np.bincount(batch, minlength=G).astype(np.float32)
    inv_cnt = 1.0 / np.maximum(cntg, 1.0)

    order = np.lexsort((dst, grp, ep, e_rank))
    trow_s, drel_s, attr_s = trow[order], drel[order], edge_attr[order]
    key = (e_rank[order] * 2 + ep[order]) * NGRP + grp[order]
    starts = np.searchsorted(key, np.arange(R * 2 * NGRP + 1))

    in_maps = []
    for r in range(R):
        ep_arrs = {}
        for e in (0, 1):
            nt = nt_l[e]
            off = off_l[e]
            nch = nch_l[e]
            choff = choff_l[e]
            tot = int(off[-1])
            chtot = int(choff[-1])
            gidx = np.zeros((128, tot * 8), np.int16)
            blob = np.zeros((128, tot * 256), ml_dtypes.float8_e4m3)
            attr4 = np.zeros((5, chtot * 128), ml_dtypes.float8_e4m3)
            attr4[4, :] = 1.0
            for g in range(NGRP):
                ntg = int(nt[g])
                if ntg == 0:
                    continue
                k = (r * 2 + e) * NGRP + g
                s0 = int(starts[k])
                n = int(counts[e, r, g])
                o = int(off[g])
                co = int(choff[g])
                idx = np.full((ntg * 128,), -1, np.int64)
                idx[:n] = trow_s[s0:s0 + n]
                if n == 0:
                    idx[0] = 0
                gidx[:, o * 8:(o + ntg) * 8] = wrap_idxs_block(idx)
                j = np.arange(n)
                t = j // 128
                p = j % 128
                dr = drel_s[s0:s0 + n].astype(np.int64)
                # S: [p_edge, d] at cols (o+t)*256 + d
                blob[p, (o + t) * 256 + dr] = 1
                # ST: [drel, p_edge] at cols (o+t)*256 + 128 + p
                blob[dr, (o + t) * 256 + 128 + p] = 1
                attr4[t % KQC, (co + t // KQC) * 128 + p] = \
                    attr_s[s0:s0 + n].astype(ml_dtypes.float8_e4m3)
            ep_arrs[e] = (gidx, blob, attr4)
        IndT = np.zeros((NPAD, G), np.float32)
        lo, hi = r * NPR, min((r + 1) * NPR, N)
        IndT[np.arange(hi - lo), batch[lo:hi]] = inv_cnt[batch[lo:hi]]
        xT_loc = np.zeros((128, NPAD), np.float32)
        xT_loc[:, 0:hi - lo] = x[lo:hi].T
        x_rb = np.zeros((R * 128, NPAD), np.float32)
        for rr in range(R):
            rlo, rhi = rr * NPR, min((rr + 1) * NPR, N)
            x_rb[rr * 128:(rr + 1) * 128, 0:rhi - rlo] = x[rlo:rhi].T
        gcnt = np.maximum(counts[:, r, :], 1).astype(np.int32).reshape(1, 2 * NGRP)
        im = {
            "gcnt": gcnt,
            "xT_loc": xT_loc.astype(ml_dtypes.float8_e4m3),
            "x_rb": x_rb.astype(ml_dtypes.float8_e4m3),
            "Wqvk": Wqvk.astype(ml_dtypes.bfloat16),
            "Ws": Ws_e.astype(ml_dtypes.bfloat16),
            "BD": BD.astype(ml_dtypes.bfloat16),
            "bv_row": np.concatenate(
                [np.zeros((NLAYER, 1, H), np.float32),
                 bv_e.reshape(NLAYER, 1, H),
                 np.zeros((NLAYER, 1, H), np.float32)], axis=2
            ).astype(ml_dtypes.bfloat16),
            "bs_col": bs_e.reshape(NLAYER, H, 1),
            "gidx0": ep_arrs[0][0], "blob0": ep_arrs[0][1], "attr0": ep_arrs[0][2],
            "gidx1": ep_arrs[1][0], "blob1": ep_arrs[1][1], "attr1": ep_arrs[1][2],
            "IndT": IndT.astype(ml_dtypes.bfloat16),
            "clinT": clinical.T.copy(),
            "Wc_h": Wc_h, "Wc_c": Wc[H:H + NCLIN],
            "bc_rep": np.tile(bc_e, (G, 1)),
        }
        in_maps.append(im)
    meta = dict(NT0=tuple(int(v) for v in nt_l[0]),
                NT1=tuple(int(v) for v in nt_l[1]),
                has_bv=has_bv, has_bs=has_bs, tab_fp8=True, dr_seg=True)
    return in_maps, meta


# ---------------------------------------------------------------------------
# device program
# ---------------------------------------------------------------------------

def build(meta):
    parts = meta.get("parts", 4)
    DT = BF16
    TDT = FP8 if meta.get("tab_fp8", True) else BF16
    NT_L = [list(meta["NT0"]), list(meta["NT1"])]
    OFF_L = [np.concatenate([[0], np.cumsum(nt)]).astype(int) for nt in NT_L]
    NCH_L = [np.ceil(np.array(nt) / KQC).astype(int) for nt in NT_L]
    CHOFF_L = [np.concatenate([[0], np.cumsum(nc)]).astype(int) for nc in NCH_L]
    TOT = [int(o[-1]) for o in OFF_L]
    CHTOT = [int(c[-1]) for c in CHOFF_L]
    TMAX = max(max(NT_L[0]), max(NT_L[1]))

    nc = bacc.Bacc("TRN2", target_bir_lowering=False, debug=False, num_devices=R)

    def din(name, shape, dt):
        return nc.dram_tensor(name, shape, dt, kind="ExternalInput").ap()

    t_xT = din("xT_loc", [128, NPAD], FP8)
    t_cnt = din("gcnt", [1, 2 * NGRP], mybir.dt.int32)
    t_xrb = din("x_rb", [R * 128, NPAD], FP8)
    t_Wqvk = din("Wqvk", [NLAYER, H, 3 * H], DT)
    t_Ws = din("Ws", [NLAYER, H, H], DT)
    t_BD = din("BD", [NLAYER, 5, KQC * H], DT)
    t_bv = din("bv_row", [NLAYER, 1, 3 * H], DT)
    t_bs = din("bs_col", [NLAYER, H, 1], F32)
    t_gidx = [din("gidx0", [128, TOT[0] * 8], I16),
              din("gidx1", [128, TOT[1] * 8], I16)]
    t_blob = [din("blob0", [128, TOT[0] * 256], FP8),
              din("blob1", [128, TOT[1] * 256], FP8)]
    t_attr = [din("attr0", [5, CHTOT[0] * 128], FP8),
              din("attr1", [5, CHTOT[1] * 128], FP8)]
    t_IndT = din("IndT", [NPAD, G], DT)
    t_clinT = din("clinT", [NCLIN, G], F32)
    t_Wc_h = din("Wc_h", [H, NCLS], F32)
    t_Wc_c = din("Wc_c", [NCLIN, NCLS], F32)
    t_bc = din("bc_rep", [G, NCLS], F32)

    t_out = nc.dram_tensor("out", [G, NCLS], F32, kind="ExternalOutput").ap()

    qv_loc = nc.dram_tensor("qv_loc", [NPAD, 2 * H], TDT).ap()
    # per-layer AG'd table pairs (separate per layer so the next layer's
    # AllGather never overwrites a table the current layer still gathers from)
    qv_tl = [None,
             [nc.dram_tensor("qv1_t0", [R * C0N, 2 * H], TDT, addr_space="Shared").ap(),
              nc.dram_tensor("qv1_t1", [R * C1N, 2 * H], TDT, addr_space="Shared").ap()],
             [nc.dram_tensor("qv2_t0", [R * C0N, 2 * H], TDT, addr_space="Shared").ap(),
              nc.dram_tensor("qv2_t1", [R * C1N, 2 * H], TDT, addr_space="Shared").ap()]]
    # layer-0 tables are built locally (x is replicated), no collective
    qv_tl[0] = [nc.dram_tensor("qv0_t0", [R * C0N, 2 * H], TDT).ap(),
                nc.dram_tensor("qv0_t1", [R * C1N, 2 * H], TDT).ap()]
    pool_in = nc.dram_tensor("pool_in", [G, H], F32).ap()
    pool_out = nc.dram_tensor("pool_out", [R * G, H], F32, addr_space="Shared").ap()

    with tile.TileContext(nc) as tc:
        import contextlib
        with contextlib.ExitStack() as ctx:
            consts = ctx.enter_context(tc.tile_pool(name="consts", bufs=1))
            hsb = ctx.enter_context(tc.tile_pool(name="hsb", bufs=1))
            h3p = ctx.enter_context(tc.tile_pool(name="h3p", bufs=1))
            ksb = ctx.enter_context(tc.tile_pool(name="ksb", bufs=1))
            stg = ctx.enter_context(tc.tile_pool(name="stg", bufs=4))
            tail = ctx.enter_context(tc.tile_pool(name="tail", bufs=2))
            edg = ctx.enter_context(tc.tile_pool(name="edg", bufs=5))
            sel = ctx.enter_context(tc.tile_pool(name="sel", bufs=8))
            edm = ctx.enter_context(tc.tile_pool(name="edm", bufs=3))
            pnode = ctx.enter_context(tc.tile_pool(name="pnode", bufs=3, space="PSUM"))
            pkq = ctx.enter_context(tc.tile_pool(name="pkq", bufs=2, space="PSUM"))
            pedge = ctx.enter_context(tc.tile_pool(name="pedge", bufs=2, space="PSUM"))
            ppool = ctx.enter_context(tc.tile_pool(name="ppool", bufs=1, space="PSUM"))

            _cid = [0]

            def load_const(src_ap, shape, dt):
                _cid[0] += 1
                t = consts.tile(shape, dt, tag=f"c{_cid[0]}_{src_ap.tensor.name}")
                nc.sync.dma_start(t[:], src_ap)
                return t

            Wqvk_t = [load_const(t_Wqvk[l], [H, 3 * H], DT) for l in range(NLAYER)]
            Ws_t = [load_const(t_Ws[l], [H, H], DT) for l in range(NLAYER)]
            BD_t = [load_const(t_BD[l], [5, KQC * H], DT) for l in range(NLAYER)]
            bv_t = ([load_const(t_bv[l], [1, 3 * H], DT) for l in range(NLAYER)]
                    if meta["has_bv"] else None)
            bs_t = ([load_const(t_bs[l], [H, 1], F32) for l in range(NLAYER)]
                    if meta["has_bs"] else None)
            ones_t = consts.tile([1, 128], DT)
            nc.vector.memset(ones_t[:], 1.0)
            gidx_t = [load_const(t_gidx[0], [128, TOT[0] * 8], I16),
                      load_const(t_gidx[1], [128, TOT[1] * 8], I16)]
            attr_t = [load_const(t_attr[0], [5, CHTOT[0] * 128], FP8),
                      load_const(t_attr[1], [5, CHTOT[1] * 128], FP8)]
            xT_t = load_const(t_xT, [128, NPAD], FP8)
            cnt_t = load_const(t_cnt, [1, 2 * NGRP], mybir.dt.int32)
            ident = consts.tile([128, 128], DT)
            make_identity(nc, ident[:])
            identf = consts.tile([128, 128], F32)
            make_identity(nc, identf[:])
            clin_t = load_const(t_clinT, [NCLIN, G], F32)
            Wch_t = load_const(t_Wc_h, [H, NCLS], F32)
            Wcc_t = load_const(t_Wc_c, [NCLIN, NCLS], F32)
            bc_t = load_const(t_bc, [G, NCLS], F32)

            # ping-pong state by layer parity
            hs_pp = [hsb.tile([128, NPAD], DT, tag="hs0", name="hs0"),
                     hsb.tile([128, NPAD], DT, tag="hs1", name="hs1")]
            k_pp = [ksb.tile([128, NGRP, H], DT, tag="k0", name="k0"),
                    ksb.tile([128, NGRP, H], DT, tag="k1", name="k1")]
            h3_pp = [h3p.tile([128, NPAD], DT, tag="h3a", name="h3a"),
                     h3p.tile([128, NPAD], DT, tag="h3b", name="h3b")]
            h3f = h3_pp[0]                             # layer-2 output parity

            def build_chunk(l, c, hsrc):
                """Local q|v|k for groups of AG-chunk c of layer l."""
                g0, g1 = (0, C0G) if c == 0 else (C0G, NGRP)
                k_sb = k_pp[l % 2]
                g = g0
                while g < g1:
                    ns = min(8, g1 - g)
                    st = stg.tile([128, 8, 2 * H], TDT, tag="st")
                    for s in range(ns):
                        gg = g + s
                        ps = pnode.tile([128, 512], F32, tag="pn")
                        nc.tensor.matmul(
                            out=ps[:, 0:3 * H],
                            lhsT=hsrc[:, gg * 128:(gg + 1) * 128],
                            rhs=Wqvk_t[l][:], start=True,
                            stop=not meta["has_bv"],
                            skip_group_check=True)
                        if meta["has_bv"]:
                            nc.tensor.matmul(
                                out=ps[:, 0:3 * H], lhsT=ones_t[:],
                                rhs=bv_t[l][:], start=False, stop=True,
                                skip_group_check=True)
                        nc.scalar.activation(st[:, s, :], ps[:, 0:2 * H],
                                             AF.Copy)
                        nc.scalar.activation(k_sb[:, gg, :],
                                             ps[:, 2 * H:3 * H], AF.Copy)
                    nc.sync.dma_start(
                        qv_loc[g * 128:(g + ns) * 128, :].rearrange(
                            "(s p) f -> p s f", p=128),
                        st[:, 0:ns, :])
                    g += ns

            def emit_ag(l, c):
                ins = qv_loc[0:C0N, :] if c == 0 else qv_loc[C0N:NPAD, :]
                nc.gpsimd.collective_compute(
                    "AllGather", OP.bypass, replica_groups=[list(range(R))],
                    ins=[ins], outs=[qv_tl[l][c][:]])

            def emit_spart(l, hsrc):
                hs = hs_pp[l % 2]
                for c0 in range(0, NPAD, 512):
                    csz = min(512, NPAD - c0)
                    pss = pnode.tile([128, 512], F32, tag="pn")
                    nc.tensor.matmul(out=pss[:, 0:csz], lhsT=Ws_t[l][:],
                                     rhs=hsrc[:, c0:c0 + csz],
                                     start=True, stop=True)
                    if meta["has_bs"]:
                        nc.scalar.activation(hs[:, c0:c0 + csz], pss[:, 0:csz],
                                             AF.Identity, bias=bs_t[l][:],
                                             scale=1.0)
                    else:
                        nc.scalar.activation(hs[:, c0:c0 + csz], pss[:, 0:csz],
                                             AF.Copy)

            def make_tail_cb(l, c_start, c_end):
                # During layer l's ep1 sweep over [c_start, c_end): as each
                # dst-group finalizes, leaky just that slice and build the
                # next layer's table rows for it, storing every 8 groups so
                # the chunk AllGather can fire the moment the sweep ends.
                st_state = {}

                def cb(g, l=l):
                    sl = slice(g * 128, (g + 1) * 128)
                    nc.vector.scalar_tensor_tensor(
                        out=h3_pp[l % 2][:, sl], in0=hs_pp[l % 2][:, sl],
                        scalar=float(SLOPE), in1=hs_pp[l % 2][:, sl],
                        op0=OP.mult, op1=OP.max)
                    if l >= NLAYER - 1:
                        return
                    s = (g - c_start) % 8
                    if s == 0:
                        st_state["st"] = stg.tile([128, 8, 2 * H], TDT,
                                                  tag="st", name="st_cb")
                        st_state["g0"] = g
                    st = st_state["st"]
                    ps = pnode.tile([128, 512], F32, tag="pn", name="pn_cb")
                    nc.tensor.matmul(
                        out=ps[:, 0:3 * H],
                        lhsT=h3_pp[l % 2][:, sl],
                        rhs=Wqvk_t[l + 1][:], start=True,
                        stop=not meta["has_bv"], skip_group_check=True)
                    if meta["has_bv"]:
                        nc.tensor.matmul(
                            out=ps[:, 0:3 * H], lhsT=ones_t[:],
                            rhs=bv_t[l + 1][:], start=False, stop=True,
                            skip_group_check=True)
                    nc.scalar.activation(st[:, s, :], ps[:, 0:2 * H], AF.Copy)
                    nc.scalar.activation(k_pp[(l + 1) % 2][:, g, :],
                                         ps[:, 2 * H:3 * H], AF.Copy)
                    if s == 7 or g == c_end - 1:
                        g0b = st_state["g0"]
                        nc.sync.dma_start(
                            qv_loc[g0b * 128:(g + 1) * 128, :].rearrange(
                                "(s p) f -> p s f", p=128),
                            st[:, 0:g + 1 - g0b, :])
                return cb

            def emit_leaky(l, c):
                sl = slice(0, C0N) if c == 0 else slice(C0N, NPAD)
                nc.vector.scalar_tensor_tensor(
                    out=h3_pp[l % 2][:, sl], in0=hs_pp[l % 2][:, sl],
                    scalar=float(SLOPE), in1=hs_pp[l % 2][:, sl],
                    op0=OP.mult, op1=OP.max)

            def emit_edges(e, l, g0, g1, after_group=None):
                nt_list = NT_L[e]
                off = OFF_L[e]
                choff = CHOFF_L[e]
                hs = hs_pp[l % 2]
                k_sb = k_pp[l % 2]
                tab = qv_tl[l][e]
                for g in range(g0, g1):
                    nt = int(nt_list[g])
                    if nt == 0:
                        if after_group is not None:
                            after_group(g)
                        continue
                    ne = nt * 128
                    o = int(off[g])
                    co = int(choff[g])
                    bt = sel.tile([128, TMAX, 256], FP8, tag="blob")
                    nc.sync.dma_start(
                        bt[:, 0:nt, :],
                        t_blob[e][:, o * 256:(o + nt) * 256].rearrange(
                            "p (t d) -> p t d", t=nt))
                    gt = edg.tile([128, TMAX, 2 * H], TDT, tag="g")
                    nreg = nc.gpsimd.value_load(
                        cnt_t[0:1, e * NGRP + g:e * NGRP + g + 1])
                    nc.gpsimd.dma_gather(
                        gt[:, 0:nt, :], tab[:],
                        gidx_t[e][:, o * 8:(o + nt) * 8],
                        ne, nreg, 2 * H, single_packet=(ne <= 512))
                    ktb = edm.tile([128, TMAX, 128], DT, tag="kt")
                    for ci in range((nt + KQC - 1) // KQC):
                        q0 = ci * KQC
                        qn = min(KQC, nt - q0)
                        pk = pkq.tile([128, KQC, 128], F32, tag="kq")
                        nc.tensor.matmul(
                            out=pk[:, 0:KQC, :],
                            lhsT=attr_t[e][0:5, (co + ci) * 128:(co + ci + 1) * 128],
                            rhs=BD_t[l][:], start=True, stop=False,
                            skip_group_check=True)
                        for t in range(q0, q0 + qn):
                            nc.tensor.matmul(
                                out=pk[:, t - q0, :],
                                lhsT=bt[:, t, 128:256],
                                rhs=k_sb[:, g, :], start=False, stop=False,
                                skip_group_check=True)
                        # q[src] folded into the same PSUM via identity matmul
                        nc.tensor.matmul(
                            out=pk[:, 0:qn, :], lhsT=ident[:],
                            rhs=gt[:, q0:q0 + qn, 0:H], start=False, stop=True,
                            skip_group_check=True)
                        nc.scalar.activation(ktb[:, q0:q0 + qn, :],
                                             pk[:, 0:qn, :], AF.Sigmoid)
                    dr = meta.get("dr_seg", False)
                    msg = edm.tile([128, TMAX, 128], FP8 if dr else DT, tag="msg")
                    nc.vector.tensor_tensor(out=msg[:, 0:nt, :],
                                            in0=ktb[:, 0:nt, :],
                                            in1=gt[:, 0:nt, H:2 * H], op=OP.mult)
                    pa = pedge.tile([128, 128], F32, tag="pa")
                    if dr:
                        npair = nt // 2
                        for t2 in range(0, npair * 2, 2):
                            nc.tensor.matmul(
                                out=pa[:], lhsT=msg[:, t2:t2 + 2, :],
                                rhs=bt[:, t2:t2 + 2, 0:128], start=(t2 == 0),
                                stop=(t2 + 2 == nt),
                                perf_mode=mybir.MatmulPerfMode.DoubleRow,
                                skip_group_check=True)
                        if nt % 2:
                            nc.tensor.matmul(out=pa[:], lhsT=msg[:, nt - 1, :],
                                             rhs=bt[:, nt - 1, 0:128],
                                             start=(nt == 1), stop=True,
                                             skip_group_check=True)
                    else:
                        for t in range(nt):
                            nc.tensor.matmul(out=pa[:], lhsT=msg[:, t, :],
                                             rhs=bt[:, t, 0:128], start=(t == 0),
                                             stop=(t == nt - 1))
                    nc.vector.tensor_tensor(
                        out=hs[:, g * 128:(g + 1) * 128],
                        in0=hs[:, g * 128:(g + 1) * 128], in1=pa[:], op=OP.add)
                    if after_group is not None:
                        after_group(g)

            # ---- layer-0 table from x: replicated input, so each rank builds
            # the FULL table locally (no collective, no exposed startup AG).
            def build_l0_chunk(c):
                g0, g1, tab, cbase = ((0, C0G, qv_tl[0][0], C0N) if c == 0
                                      else (C0G, NGRP, qv_tl[0][1], C1N))
                for rb in range(R):
                    g = g0
                    while g < g1:
                        ns = min(8, g1 - g)
                        lh = stg.tile([128, 1024], FP8, tag="lh")
                        nc.sync.dma_start(
                            lh[:, 0:ns * 128],
                            t_xrb[rb * 128:(rb + 1) * 128,
                                  g * 128:(g + ns) * 128])
                        st = stg.tile([128, 8, 2 * H], TDT, tag="st")
                        for s in range(0, ns, 2):
                            n2 = min(2, ns - s)
                            ps = pnode.tile([128, 512], F32, tag="pn")
                            for u in range(n2):
                                nc.tensor.matmul(
                                    out=ps[:, u * 256:u * 256 + 2 * H],
                                    lhsT=lh[:, (s + u) * 128:(s + u + 1) * 128],
                                    rhs=Wqvk_t[0][:, 0:2 * H], start=True,
                                    stop=not meta["has_bv"],
                                    skip_group_check=True)
                                if meta["has_bv"]:
                                    nc.tensor.matmul(
                                        out=ps[:, u * 256:u * 256 + 2 * H],
                                        lhsT=ones_t[:],
                                        rhs=bv_t[0][0:1, 0:2 * H],
                                        start=False, stop=True,
                                        skip_group_check=True)
                            if (s // 2) % 2 == 0:
                                nc.scalar.activation(st[:, s:s + n2, :],
                                                     ps[:, 0:n2 * 256], AF.Copy)
                            else:
                                nc.vector.tensor_copy(st[:, s:s + n2, :],
                                                      ps[:, 0:n2 * 256])
                        r0 = rb * cbase + (g - g0) * 128
                        nc.sync.dma_start(
                            tab[r0:r0 + ns * 128, :].rearrange(
                                "(s p) f -> p s f", p=128),
                            st[:, 0:ns, :])
                        g += ns
            build_l0_chunk(0)
            # local k table + s-part for layer 0 from the local x slice
            for g in range(NGRP):
                psk = pnode.tile([128, 512], F32, tag="pn")
                nc.tensor.matmul(out=psk[:, 0:H],
                                 lhsT=xT_t[:, g * 128:(g + 1) * 128],
                                 rhs=Wqvk_t[0][:, 2 * H:3 * H],
                                 start=True, stop=True, skip_group_check=True)
                nc.vector.tensor_copy(k_pp[0][:, g, :], psk[:, 0:H])
            emit_spart(0, xT_t)

            for _ in range(5):
                gz = edg.tile([128, TMAX, 2 * H], TDT, tag="g")
                nc.vector.memset(gz[:], 0.0)

            pp = ppool.tile([G, H], F32)
            indc = consts.tile([128, NGRP, G], DT)
            nc.sync.dma_start(
                indc[:], t_IndT[:].rearrange("(c p) g -> p c g", p=128))

            def make_pool_cb(l):
                def cb(g, l=l):
                    sl = slice(g * 128, (g + 1) * 128)
                    nc.vector.scalar_tensor_tensor(
                        out=h3_pp[l % 2][:, sl], in0=hs_pp[l % 2][:, sl],
                        scalar=float(SLOPE), in1=hs_pp[l % 2][:, sl],
                        op0=OP.mult, op1=OP.max)
                    trp = pedge.tile([128, 128], DT, tag="pa", name="trp_cb")
                    nc.tensor.transpose(out=trp[:], in_=h3f[:, sl],
                                        identity=ident[:])
                    hnode = tail.tile([128, 128], DT, tag="hnode",
                                      name="hnode_cb")
                    nc.vector.tensor_copy(hnode[:], trp[:])
                    nc.tensor.matmul(out=pp[:], lhsT=indc[:, g, :],
                                     rhs=hnode[:],
                                     start=(g == 0), stop=(g == NGRP - 1))
                return cb

            def emit_pool_part(c0g, c1g):
                for c in range(c0g, c1g):
                    trp = pedge.tile([128, 128], DT, tag="pa")
                    nc.tensor.transpose(out=trp[:],
                                        in_=h3f[:, c * 128:(c + 1) * 128],
                                        identity=ident[:])
                    hnode = tail.tile([128, 128], DT, tag="hnode")
                    nc.vector.tensor_copy(hnode[:], trp[:])
                    nc.tensor.matmul(out=pp[:],
                                     lhsT=indc[:, c, :],
                                     rhs=hnode[:],
                                     start=(c == 0), stop=(c == NGRP - 1))

            for l in range(NLAYER):
                hsrc_next = h3_pp[l % 2]
                last = l >= NLAYER - 1
                if parts >= 3 and not last:
                    cb0 = make_tail_cb(l, 0, C0G)
                    cb1 = make_tail_cb(l, C0G, NGRP)
                elif parts >= 4 and last:
                    cb0 = cb1 = make_pool_cb(l)
                else:
                    cb0 = cb1 = None
                if parts >= 2:
                    if l == 0:
                        # layer 0 consumes no AllGather (local tables), so run
                        # both epochs' first halves up front and fire the hook
                        # at ~50% -- the next layer's AG0 starts much earlier.
                        emit_edges(0, l, 0, 12)
                        build_l0_chunk(1)
                        emit_edges(0, l, 12, C0G)
                        emit_edges(1, l, 0, C0G, after_group=cb0)
                    else:
                        # full epoch-0 sweep first (its table is ready), then
                        # the first ep1 half; this keeps ep0 work ahead of any
                        # ep1 stall on the second table chunk's AllGather.
                        emit_edges(0, l, 0, NGRP)
                        emit_edges(1, l, 0, C0G, after_group=cb0)
                # h3 chunk-0 final: fire next layer's chunk-0 AllGather (the
                # per-group callback already built and stored the rows)
                if not last:
                    if parts < 2:
                        emit_leaky(l, 0)
                        build_chunk(l + 1, 0, hsrc_next)
                    emit_ag(l + 1, 0)
                elif cb0 is None:
                    emit_leaky(l, 0)
                if parts >= 2:
                    if l == 0:
                        emit_edges(0, l, C0G, NGRP)
                    emit_edges(1, l, C0G, NGRP, after_group=cb1)
                if parts < 3:
                    break
                if not last:
                    emit_ag(l + 1, 1)
                    emit_spart(l + 1, hsrc_next)
                elif cb1 is None:
                    emit_leaky(l, 1)
            if parts < 4:
                z_dbg = tail.tile([G, NCLS], F32, tag="zsb")
                nc.vector.tensor_copy(z_dbg[:], hs_pp[0][0:G, 0:NCLS])
                nc.sync.dma_start(t_out[:], z_dbg[:])
            else:
                pool_sb = tail.tile([G, H], F32, tag="poolsb")
                nc.vector.tensor_copy(pool_sb[:], pp[:])
                nc.sync.dma_start(pool_in[:], pool_sb[:])
                nc.gpsimd.collective_compute(
                    "AllGather", OP.bypass, replica_groups=[list(range(R))],
                    ins=[pool_in[:]], outs=[pool_out[:]])
                pr = tail.tile([G, R, H], F32, tag="pr")
                nc.sync.dma_start(pr[:], pool_out[:].rearrange("(r g) h -> g r h", r=R))
                pooled = tail.tile([G, H], F32, tag="pooled")
                nc.vector.tensor_tensor(out=pooled[:], in0=pr[:, 0, :], in1=pr[:, 1, :],
                                        op=OP.add)
                for r in range(2, R):
                    nc.vector.tensor_tensor(out=pooled[:], in0=pooled[:],
                                            in1=pr[:, r, :], op=OP.add)
                ptp = pedge.tile([H, G], F32, tag="pa")
                nc.tensor.transpose(out=ptp[:], in_=pooled[:], identity=identf[0:G, 0:G])
                pooledT = tail.tile([H, G], F32, tag="pooledT")
                nc.vector.tensor_copy(pooledT[:], ptp[:])
                zp = pedge.tile([G, NCLS], F32, tag="pa")
                nc.tensor.matmul(out=zp[:], lhsT=pooledT[:], rhs=Wch_t[:],
                                 start=True, stop=False)
                nc.tensor.matmul(out=zp[:], lhsT=clin_t[:], rhs=Wcc_t[:],
                                 start=False, stop=True)
                z_sb = tail.tile([G, NCLS], F32, tag="zsb")
                nc.vector.tensor_tensor(out=z_sb[:], in0=zp[:], in1=bc_t[:],
                                        op=OP.add)
                nc.sync.dma_start(t_out[:], z_sb[:])

    nc.compile()
    return nc


# ---------------------------------------------------------------------------

_CACHE = {}


def kernel(**inputs):
    in_maps, meta = prep(inputs)
    key = tuple(sorted((k, v) for k, v in meta.items()))
    if key not in _CACHE:
        _CACHE[key] = build(meta)
    nc = _CACHE[key]
    res = run_bass_kernel_spmd(nc, in_maps, list(range(R)))
    return np.asarray(res.results[0]["out"], np.float32)


def kernel_profiled(**inputs):
    """Like kernel() but also returns (exec_time_ns, trace_path)."""
    in_maps, meta = prep(inputs)
    key = tuple(sorted((k, v) for k, v in meta.items()))
    if key not in _CACHE:
        _CACHE[key] = build(meta)
    nc = _CACHE[key]
    res = run_bass_kernel_spmd(nc, in_maps, list(range(R)), trace=True)
    out = np.asarray(res.results[0]["out"], np.float32)
    trace_path = None
    if res.instructions_and_trace is not None:
        trace_path = res.instructions_and_trace[1]
    return out, res.exec_time_ns, trace_path


if __name__ == "__main__":
    pass


# revision 67
# speedup vs baseline: 1.0102x; 1.0017x over previous
"""Trainium2 Bass kernel for a 3-layer ResGatedGraphConv GNN (ClinicalGatedGCN).

Strategy (8 NeuronCores, SPMD), v3:
  - Nodes partitioned into 8 contiguous ranges of 6250 (padded to 6272 = 49
    groups of 128). Edges assigned to the rank owning their dst node, sorted
    by (src-chunk epoch, dst-group, dst) on the host.
  - The [q|v] node table is built DISTRIBUTED: each rank computes q,v (and k)
    only for its OWN nodes from its SBUF-resident h (one fused matmul per
    128-node group against [Wq|Wv|Wk]), stores the [q|v] rows to local DRAM,
    and AllGathers the table in two chunks (groups 0:24 -> qv_t0, 24:49 ->
    qv_t1; both tables stay under the int16 gather-index limit). This removes
    the per-layer h AllGather, all h re-reads, and ~7/8 of the table matmuls
    and PSUM->SBUF copies of the replicated design.
  - One dma_gather per (epoch, dst-group) fetches the src rows of [q|v].
    k[dst] is expanded on the PE via a host-shipped 0/1 selector ST; the
    segment-sum over dst is a PE matmul against selector S. S and ST are
    packed in ONE fp8 blob per (epoch, group) (0/1 is exact in fp8; matmul
    operands may mix fp8 with bf16), halving selector DMA traffic.
  - The edge-embedding rank-1 term (attr x We + gate bias) is batched: one
    K=5 matmul per 4-tile PSUM chunk against a block-diagonal [5, 4H]
    constant, with per-chunk attr columns shipped once as an fp8 constant.
  - BatchNorm folding: A = gamma/sqrt(var+eps) is folded into the NEXT
    layer's weights (and the classifier); B is folded into effective biases.
    The on-device h update is a single fused leaky-ReLU
    (scalar_tensor_tensor max(x, slope*x)).
  - Mean-pool per graph is a matmul against a host-built indicator with
    1/cnt folded in; partial pools are AllGather'd and summed; the tiny
    classifier runs on every core.
"""

import numpy as np
import ml_dtypes

import concourse.bacc as bacc
import concourse.bass as bass
import concourse.mybir as mybir
import concourse.tile as tile
from concourse.bass_utils import run_bass_kernel_spmd
from concourse.masks import make_identity

F32 = mybir.dt.float32
BF16 = mybir.dt.bfloat16
FP8 = mybir.dt.float8e4
I16 = mybir.dt.int16
AF = mybir.ActivationFunctionType
OP = mybir.AluOpType

# ---------------- problem constants (hardcoded per spec) ----------------
N, E, H, G, NCLIN, NCLS = 50000, 800000, 128, 64, 16, 2
NLAYER = 3
EPS = 1e-5
SLOPE = 0.01
R = 8                      # ranks / NeuronCores

NPR = (N + R - 1) // R     # real nodes per rank (6250)
NGRP = (NPR + 127) // 128  # 128-node groups per rank (49)
NPAD = NGRP * 128          # padded nodes per rank (6272)
C0G = 24                   # groups in AG chunk 0
C1G = NGRP - C0G           # groups in AG chunk 1 (25)
C0N = C0G * 128            # 3072 rows
C1N = C1G * 128            # 3200 rows
KQC = 4                    # psum chunk size in edge tiles (1 PSUM bank)


def wrap_idxs_block(idx):
    """Wrap one gather call's indices: idx j -> [j%16, j//16], tiled to 128 parts."""
    n = len(idx)
    assert n % 16 == 0
    w = np.asarray(idx, np.int16).reshape(n // 16, 16).T
    return np.tile(w, (8, 1))


# ---------------------------------------------------------------------------
# host-side preprocessing
# ---------------------------------------------------------------------------

def prep(inputs):
    x = np.asarray(inputs["x"], np.float32)
    edge_index = np.asarray(inputs["edge_index"])
    edge_attr = np.asarray(inputs["edge_attr"], np.float32)[:, 0]
    batch = np.asarray(inputs["batch"]).astype(np.int64)
    clinical = np.asarray(inputs["clinical"], np.float32)
    Wk, bk = np.asarray(inputs["Wk"], np.float32), np.asarray(inputs["bk"], np.float32)
    Wq, bq = np.asarray(inputs["Wq"], np.float32), np.asarray(inputs["bq"], np.float32)
    Wv, bv = np.asarray(inputs["Wv"], np.float32), np.asarray(inputs["bv"], np.float32)
    Ws, bs = np.asarray(inputs["Ws"], np.float32), np.asarray(inputs["bs"], np.float32)
    We, be = np.asarray(inputs["We"], np.float32), np.asarray(inputs["be"], np.float32)
    gamma = np.asarray(inputs["gamma"], np.float32)
    beta = np.asarray(inputs["beta"], np.float32)
    rmean = np.asarray(inputs["rmean"], np.float32)
    rvar = np.asarray(inputs["rvar"], np.float32)
    Wc, bc = np.asarray(inputs["Wc"], np.float32), np.asarray(inputs["bc"], np.float32)

    src = edge_index[0].astype(np.int64)
    dst = edge_index[1].astype(np.int64)

    # BN folded: true h_out = A*leaky(pre) + B; device h3 = leaky(pre).
    A = gamma / np.sqrt(rvar + EPS)          # [3, H]
    B = beta - rmean * A                     # [3, H]

    # effective weights: fold diag(A[l-1]) into layer-l input maps, and the
    # B[l-1] offset into layer-l biases.
    Wq_e = np.stack([Wq[l] * (A[l - 1][:, None] if l else 1.0) for l in range(NLAYER)])
    Wk_e = np.stack([Wk[l] * (A[l - 1][:, None] if l else 1.0) for l in range(NLAYER)])
    Wv_e = np.stack([Wv[l] * (A[l - 1][:, None] if l else 1.0) for l in range(NLAYER)])
    Ws_e = np.stack([Ws[l] * (A[l - 1][:, None] if l else 1.0) for l in range(NLAYER)])
    bgate = np.stack([bk[l] + bq[l] + be[l]
                      + (B[l - 1] @ (Wk[l] + Wq[l]) if l else 0.0)
                      for l in range(NLAYER)])
    bv_e = np.stack([bv[l] + (B[l - 1] @ Wv[l] if l else 0.0) for l in range(NLAYER)])
    bs_e = np.stack([bs[l] + (B[l - 1] @ Ws[l] if l else 0.0) for l in range(NLAYER)])
    Wc_h = Wc[0:H] * A[2][:, None]
    bc_e = bc + B[2] @ Wc[0:H]

    Wqvk = np.concatenate([Wq_e, Wv_e, Wk_e], axis=2)     # [3, H, 3H]

    # block-diagonal edge-term constant: rows 0..3 carry We on diag block,
    # row 4 carries the gate bias (broadcast to each tile block).
    BD = np.zeros((NLAYER, 5, KQC * H), np.float32)
    for l in range(NLAYER):
        for t in range(KQC):
            BD[l, t, t * H:(t + 1) * H] = We[l, 0, :]
            BD[l, 4, t * H:(t + 1) * H] = bgate[l]

    has_bv = bool(np.any(bv_e != 0))
    has_bs = bool(np.any(bs_e != 0))

    # ---- edge structure ----
    e_rank = dst // NPR
    dst_local = dst - e_rank * NPR
    grp = dst_local // 128
    drel = dst_local % 128
    rs = src // NPR
    lr = src - rs * NPR
    ep = (lr >= C0N).astype(np.int64)
    trow = np.where(ep == 0, rs * C0N + lr, rs * C1N + (lr - C0N))

    counts = np.zeros((2, R, NGRP), np.int64)
    np.add.at(counts, (ep, e_rank, grp), 1)
    nt_l = [np.ceil(counts[e].max(axis=0) / 128).astype(int) for e in (0, 1)]
    off_l = [np.concatenate([[0], np.cumsum(nt)]).astype(int) for nt in nt_l]
    nch_l = [np.ceil(nt / KQC).astype(int) for nt in nt_l]
    choff_l = [np.concatenate([[0], np.cumsum(nc)]).astype(int) for nc in nch_l]

    cntg = np.bincount(batch, minlength=G).astype(np.float32)
    inv_cnt = 1.0 / np.maximum(cntg, 1.0)

    order = np.lexsort((dst, grp, ep, e_rank))
    trow_s, drel_s, attr_s = trow[order], drel[order], edge_attr[order]
    key = (e_rank[order] * 2 + ep[order]) * NGRP + grp[order]
    starts = np.searchsorted(key, np.arange(R * 2 * NGRP + 1))

    in_maps = []
    for r in range(R):
        ep_arrs = {}
        for e in (0, 1):
            nt = nt_l[e]
            off = off_l[e]
            nch = nch_l[e]
            choff = choff_l[e]
            tot = int(off[-1])
            chtot = int(choff[-1])
            gidx = np.zeros((128, tot * 8), np.int16)
            blob = np.zeros((128, tot * 256), ml_dtypes.float8_e4m3)
            attr4 = np.zeros((5, chtot * 128), ml_dtypes.float8_e4m3)
            attr4[4, :] = 1.0
            for g in range(NGRP):
                ntg = int(nt[g])
                if ntg == 0:
                    continue
                k = (r * 2 + e) * NGRP + g
                s0 = int(starts[k])
                n = int(counts[e, r, g])
                o = int(off[g])
                co = int(choff[g])
                idx = np.full((ntg * 128,), -1, np.int64)
                idx[:n] = trow_s[s0:s0 + n]
                if n == 0:
                    idx[0] = 0
                gidx[:, o * 8:(o + ntg) * 8] = wrap_idxs_block(idx)
                j = np.arange(n)
                t = j // 128
                p = j % 128
                dr = drel_s[s0:s0 + n].astype(np.int64)
                # S: [p_edge, d] at cols (o+t)*256 + d
                blob[p, (o + t) * 256 + dr] = 1
                # ST: [drel, p_edge] at cols (o+t)*256 + 128 + p
                blob[dr, (o + t) * 256 + 128 + p] = 1
                attr4[t % KQC, (co + t // KQC) * 128 + p] = \
                    attr_s[s0:s0 + n].astype(ml_dtypes.float8_e4m3)
            ep_arrs[e] = (gidx, blob, attr4)
        IndT = np.zeros((NPAD, G), np.float32)
        lo, hi = r * NPR, min((r + 1) * NPR, N)
        IndT[np.arange(hi - lo), batch[lo:hi]] = inv_cnt[batch[lo:hi]]
        xT_loc = np.zeros((128, NPAD), np.float32)
        xT_loc[:, 0:hi - lo] = x[lo:hi].T
        x_rb = np.zeros((R * 128, NPAD), np.float32)
        for rr in range(R):
            rlo, rhi = rr * NPR, min((rr + 1) * NPR, N)
            x_rb[rr * 128:(rr + 1) * 128, 0:rhi - rlo] = x[rlo:rhi].T
        gcnt = np.maximum(counts[:, r, :], 1).astype(np.int32).reshape(1, 2 * NGRP)
        im = {
            "gcnt": gcnt,
            "xT_loc": xT_loc.astype(ml_dtypes.float8_e4m3),
            "x_rb": x_rb.astype(ml_dtypes.float8_e4m3),
            "Wqvk": Wqvk.astype(ml_dtypes.bfloat16),
            "Ws": Ws_e.astype(ml_dtypes.bfloat16),
            "BD": BD.astype(ml_dtypes.bfloat16),
            "bv_row": np.concatenate(
                [np.zeros((NLAYER, 1, H), np.float32),
                 bv_e.reshape(NLAYER, 1, H),
                 np.zeros((NLAYER, 1, H), np.float32)], axis=2
            ).astype(ml_dtypes.bfloat16),
            "bs_col": bs_e.reshape(NLAYER, H, 1),
            "gidx0": ep_arrs[0][0], "blob0": ep_arrs[0][1], "attr0": ep_arrs[0][2],
            "gidx1": ep_arrs[1][0], "blob1": ep_arrs[1][1], "attr1": ep_arrs[1][2],
            "IndT": IndT.astype(ml_dtypes.bfloat16),
            "clinT": clinical.T.copy(),
            "Wc_h": Wc_h, "Wc_c": Wc[H:H + NCLIN],
            "bc_rep": np.tile(bc_e, (G, 1)),
        }
        in_maps.append(im)
    meta = dict(NT0=tuple(int(v) for v in nt_l[0]),
                NT1=tuple(int(v) for v in nt_l[1]),
                has_bv=has_bv, has_bs=has_bs, tab_fp8=True, dr_seg=True)
    return in_maps, meta


# ---------------------------------------------------------------------------
# device program
# ---------------------------------------------------------------------------

def build(meta):
    parts = meta.get("parts", 4)
    DT = BF16
    TDT = FP8 if meta.get("tab_fp8", True) else BF16
    NT_L = [list(meta["NT0"]), list(meta["NT1"])]
    OFF_L = [np.concatenate([[0], np.cumsum(nt)]).astype(int) for nt in NT_L]
    NCH_L = [np.ceil(np.array(nt) / KQC).astype(int) for nt in NT_L]
    CHOFF_L = [np.concatenate([[0], np.cumsum(nc)]).astype(int) for nc in NCH_L]
    TOT = [int(o[-1]) for o in OFF_L]
    CHTOT = [int(c[-1]) for c in CHOFF_L]
    TMAX = max(max(NT_L[0]), max(NT_L[1]))

    nc = bacc.Bacc("TRN2", target_bir_lowering=False, debug=False, num_devices=R)

    def din(name, shape, dt):
        return nc.dram_tensor(name, shape, dt, kind="ExternalInput").ap()

    t_xT = din("xT_loc", [128, NPAD], FP8)
    t_cnt = din("gcnt", [1, 2 * NGRP], mybir.dt.int32)
    t_xrb = din("x_rb", [R * 128, NPAD], FP8)
    t_Wqvk = din("Wqvk", [NLAYER, H, 3 * H], DT)
    t_Ws = din("Ws", [NLAYER, H, H], DT)
    t_BD = din("BD", [NLAYER, 5, KQC * H], DT)
    t_bv = din("bv_row", [NLAYER, 1, 3 * H], DT)
    t_bs = din("bs_col", [NLAYER, H, 1], F32)
    t_gidx = [din("gidx0", [128, TOT[0] * 8], I16),
              din("gidx1", [128, TOT[1] * 8], I16)]
    t_blob = [din("blob0", [128, TOT[0] * 256], FP8),
              din("blob1", [128, TOT[1] * 256], FP8)]
    t_attr = [din("attr0", [5, CHTOT[0] * 128], FP8),
              din("attr1", [5, CHTOT[1] * 128], FP8)]
    t_IndT = din("IndT", [NPAD, G], DT)
    t_clinT = din("clinT", [NCLIN, G], F32)
    t_Wc_h = din("Wc_h", [H, NCLS], F32)
    t_Wc_c = din("Wc_c", [NCLIN, NCLS], F32)
    t_bc = din("bc_rep", [G, NCLS], F32)

    t_out = nc.dram_tensor("out", [G, NCLS], F32, kind="ExternalOutput").ap()

    qv_loc = nc.dram_tensor("qv_loc", [NPAD, 2 * H], TDT).ap()
    # per-layer AG'd table pairs (separate per layer so the next layer's
    # AllGather never overwrites a table the current layer still gathers from)
    qv_tl = [None,
             [nc.dram_tensor("qv1_t0", [R * C0N, 2 * H], TDT, addr_space="Shared").ap(),
              nc.dram_tensor("qv1_t1", [R * C1N, 2 * H], TDT, addr_space="Shared").ap()],
             [nc.dram_tensor("qv2_t0", [R * C0N, 2 * H], TDT, addr_space="Shared").ap(),
              nc.dram_tensor("qv2_t1", [R * C1N, 2 * H], TDT, addr_space="Shared").ap()]]
    # layer-0 tables are built locally (x is replicated), no collective
    qv_tl[0] = [nc.dram_tensor("qv0_t0", [R * C0N, 2 * H], TDT).ap(),
                nc.dram_tensor("qv0_t1", [R * C1N, 2 * H], TDT).ap()]
    pool_in = nc.dram_tensor("pool_in", [G, H], F32).ap()
    pool_out = nc.dram_tensor("pool_out", [R * G, H], F32, addr_space="Shared").ap()

    with tile.TileContext(nc) as tc:
        import contextlib
        with contextlib.ExitStack() as ctx:
            consts = ctx.enter_context(tc.tile_pool(name="consts", bufs=1))
            hsb = ctx.enter_context(tc.tile_pool(name="hsb", bufs=1))
            h3p = ctx.enter_context(tc.tile_pool(name="h3p", bufs=1))
            ksb = ctx.enter_context(tc.tile_pool(name="ksb", bufs=1))
            stg = ctx.enter_context(tc.tile_pool(name="stg", bufs=4))
            tail = ctx.enter_context(tc.tile_pool(name="tail", bufs=2))
            edg = ctx.enter_context(tc.tile_pool(name="edg", bufs=5))
            sel = ctx.enter_context(tc.tile_pool(name="sel", bufs=8))
            edm = ctx.enter_context(tc.tile_pool(name="edm", bufs=3))
            pnode = ctx.enter_context(tc.tile_pool(name="pnode", bufs=3, space="PSUM"))
            pkq = ctx.enter_context(tc.tile_pool(name="pkq", bufs=2, space="PSUM"))
            pedge = ctx.enter_context(tc.tile_pool(name="pedge", bufs=2, space="PSUM"))
            ppool = ctx.enter_context(tc.tile_pool(name="ppool", bufs=1, space="PSUM"))

            _cid = [0]

            def load_const(src_ap, shape, dt):
                _cid[0] += 1
                t = consts.tile(shape, dt, tag=f"c{_cid[0]}_{src_ap.tensor.name}")
                nc.sync.dma_start(t[:], src_ap)
                return t

            Wqvk_t = [load_const(t_Wqvk[l], [H, 3 * H], DT) for l in range(NLAYER)]
            Ws_t = [load_const(t_Ws[l], [H, H], DT) for l in range(NLAYER)]
            BD_t = [load_const(t_BD[l], [5, KQC * H], DT) for l in range(NLAYER)]
            bv_t = ([load_const(t_bv[l], [1, 3 * H], DT) for l in range(NLAYER)]
                    if meta["has_bv"] else None)
            bs_t = ([load_const(t_bs[l], [H, 1], F32) for l in range(NLAYER)]
                    if meta["has_bs"] else None)
            ones_t = consts.tile([1, 128], DT)
            nc.vector.memset(ones_t[:], 1.0)
            gidx_t = [load_const(t_gidx[0], [128, TOT[0] * 8], I16),
                      load_const(t_gidx[1], [128, TOT[1] * 8], I16)]
            attr_t = [load_const(t_attr[0], [5, CHTOT[0] * 128], FP8),
                      load_const(t_attr[1], [5, CHTOT[1] * 128], FP8)]
            xT_t = load_const(t_xT, [128, NPAD], FP8)
            cnt_t = load_const(t_cnt, [1, 2 * NGRP], mybir.dt.int32)
            ident = consts.tile([128, 128], DT)
            make_identity(nc, ident[:])
            identf = consts.tile([128, 128], F32)
            make_identity(nc, identf[:])
            clin_t = load_const(t_clinT, [NCLIN, G], F32)
            Wch_t = load_const(t_Wc_h, [H, NCLS], F32)
            Wcc_t = load_const(t_Wc_c, [NCLIN, NCLS], F32)
            bc_t = load_const(t_bc, [G, NCLS], F32)

            # ping-pong state by layer parity
            hs_pp = [hsb.tile([128, NPAD], DT, tag="hs0", name="hs0"),
                     hsb.tile([128, NPAD], DT, tag="hs1", name="hs1")]
            k_pp = [ksb.tile([128, NGRP, H], DT, tag="k0", name="k0"),
                    ksb.tile([128, NGRP, H], DT, tag="k1", name="k1")]
            h3_pp = [h3p.tile([128, NPAD], DT, tag="h3a", name="h3a"),
                     h3p.tile([128, NPAD], DT, tag="h3b", name="h3b")]
            h3f = h3_pp[0]                             # layer-2 output parity

            def build_chunk(l, c, hsrc):
                """Local q|v|k for groups of AG-chunk c of layer l."""
                g0, g1 = (0, C0G) if c == 0 else (C0G, NGRP)
                k_sb = k_pp[l % 2]
                g = g0
                while g < g1:
                    ns = min(8, g1 - g)
                    st = stg.tile([128, 8, 2 * H], TDT, tag="st")
                    for s in range(ns):
                        gg = g + s
                        ps = pnode.tile([128, 512], F32, tag="pn")
                        nc.tensor.matmul(
                            out=ps[:, 0:3 * H],
                            lhsT=hsrc[:, gg * 128:(gg + 1) * 128],
                            rhs=Wqvk_t[l][:], start=True,
                            stop=not meta["has_bv"],
                            skip_group_check=True)
                        if meta["has_bv"]:
                            nc.tensor.matmul(
                                out=ps[:, 0:3 * H], lhsT=ones_t[:],
                                rhs=bv_t[l][:], start=False, stop=True,
                                skip_group_check=True)
                        nc.scalar.activation(st[:, s, :], ps[:, 0:2 * H],
                                             AF.Copy)
                        nc.scalar.activation(k_sb[:, gg, :],
                                             ps[:, 2 * H:3 * H], AF.Copy)
                    nc.sync.dma_start(
                        qv_loc[g * 128:(g + ns) * 128, :].rearrange(
                            "(s p) f -> p s f", p=128),
                        st[:, 0:ns, :])
                    g += ns

            def emit_ag(l, c):
                ins = qv_loc[0:C0N, :] if c == 0 else qv_loc[C0N:NPAD, :]
                nc.gpsimd.collective_compute(
                    "AllGather", OP.bypass, replica_groups=[list(range(R))],
                    ins=[ins], outs=[qv_tl[l][c][:]])

            def emit_spart(l, hsrc):
                hs = hs_pp[l % 2]
                for c0 in range(0, NPAD, 512):
                    csz = min(512, NPAD - c0)
                    pss = pnode.tile([128, 512], F32, tag="pn")
                    nc.tensor.matmul(out=pss[:, 0:csz], lhsT=Ws_t[l][:],
                                     rhs=hsrc[:, c0:c0 + csz],
                                     start=True, stop=True)
                    if meta["has_bs"]:
                        nc.scalar.activation(hs[:, c0:c0 + csz], pss[:, 0:csz],
                                             AF.Identity, bias=bs_t[l][:],
                                             scale=1.0)
                    else:
                        nc.scalar.activation(hs[:, c0:c0 + csz], pss[:, 0:csz],
                                             AF.Copy)

            def make_tail_cb(l, c_start, c_end):
                # During layer l's ep1 sweep over [c_start, c_end): as each
                # dst-group finalizes, leaky just that slice and build the
                # next layer's table rows for it, storing every 8 groups so
                # the chunk AllGather can fire the moment the sweep ends.
                st_state = {}

                def cb(g, l=l):
                    sl = slice(g * 128, (g + 1) * 128)
                    nc.vector.scalar_tensor_tensor(
                        out=h3_pp[l % 2][:, sl], in0=hs_pp[l % 2][:, sl],
                        scalar=float(SLOPE), in1=hs_pp[l % 2][:, sl],
                        op0=OP.mult, op1=OP.max)
                    if l >= NLAYER - 1:
                        return
                    s = (g - c_start) % 8
                    if s == 0:
                        st_state["st"] = stg.tile([128, 8, 2 * H], TDT,
                                                  tag="st", name="st_cb")
                        st_state["g0"] = g
                    st = st_state["st"]
                    ps = pnode.tile([128, 512], F32, tag="pn", name="pn_cb")
                    nc.tensor.matmul(
                        out=ps[:, 0:3 * H],
                        lhsT=h3_pp[l % 2][:, sl],
                        rhs=Wqvk_t[l + 1][:], start=True,
                        stop=not meta["has_bv"], skip_group_check=True)
                    if meta["has_bv"]:
                        nc.tensor.matmul(
                            out=ps[:, 0:3 * H], lhsT=ones_t[:],
                            rhs=bv_t[l + 1][:], start=False, stop=True,
                            skip_group_check=True)
                    nc.scalar.activation(st[:, s, :], ps[:, 0:2 * H], AF.Copy)
                    nc.vector.tensor_copy(k_pp[(l + 1) % 2][:, g, :],
                                          ps[:, 2 * H:3 * H])
                    if s == 7 or g == c_end - 1:
                        g0b = st_state["g0"]
                        nc.sync.dma_start(
                            qv_loc[g0b * 128:(g + 1) * 128, :].rearrange(
                                "(s p) f -> p s f", p=128),
                            st[:, 0:g + 1 - g0b, :])
                return cb

            def emit_leaky(l, c):
                sl = slice(0, C0N) if c == 0 else slice(C0N, NPAD)
                nc.vector.scalar_tensor_tensor(
                    out=h3_pp[l % 2][:, sl], in0=hs_pp[l % 2][:, sl],
                    scalar=float(SLOPE), in1=hs_pp[l % 2][:, sl],
                    op0=OP.mult, op1=OP.max)

            def emit_edges(e, l, g0, g1, after_group=None):
                nt_list = NT_L[e]
                off = OFF_L[e]
                choff = CHOFF_L[e]
                hs = hs_pp[l % 2]
                k_sb = k_pp[l % 2]
                tab = qv_tl[l][e]
                for g in range(g0, g1):
                    nt = int(nt_list[g])
                    if nt == 0:
                        if after_group is not None:
                            after_group(g)
                        continue
                    ne = nt * 128
                    o = int(off[g])
                    co = int(choff[g])
                    bt = sel.tile([128, TMAX, 256], FP8, tag="blob")
                    nc.sync.dma_start(
                        bt[:, 0:nt, :],
                        t_blob[e][:, o * 256:(o + nt) * 256].rearrange(
                            "p (t d) -> p t d", t=nt))
                    gt = edg.tile([128, TMAX, 2 * H], TDT, tag="g")
                    nreg = nc.gpsimd.value_load(
                        cnt_t[0:1, e * NGRP + g:e * NGRP + g + 1])
                    nc.gpsimd.dma_gather(
                        gt[:, 0:nt, :], tab[:],
                        gidx_t[e][:, o * 8:(o + nt) * 8],
                        ne, nreg, 2 * H, single_packet=(ne <= 512))
                    ktb = edm.tile([128, TMAX, 128], DT, tag="kt")
                    for ci in range((nt + KQC - 1) // KQC):
                        q0 = ci * KQC
                        qn = min(KQC, nt - q0)
                        pk = pkq.tile([128, KQC, 128], F32, tag="kq")
                        nc.tensor.matmul(
                            out=pk[:, 0:KQC, :],
                            lhsT=attr_t[e][0:5, (co + ci) * 128:(co + ci + 1) * 128],
                            rhs=BD_t[l][:], start=True, stop=False,
                            skip_group_check=True)
                        for t in range(q0, q0 + qn):
                            nc.tensor.matmul(
                                out=pk[:, t - q0, :],
                                lhsT=bt[:, t, 128:256],
                                rhs=k_sb[:, g, :], start=False, stop=False,
                                skip_group_check=True)
                        # q[src] folded into the same PSUM via identity matmul
                        nc.tensor.matmul(
                            out=pk[:, 0:qn, :], lhsT=ident[:],
                            rhs=gt[:, q0:q0 + qn, 0:H], start=False, stop=True,
                            skip_group_check=True)
                        nc.scalar.activation(ktb[:, q0:q0 + qn, :],
                                             pk[:, 0:qn, :], AF.Sigmoid)
                    dr = meta.get("dr_seg", False)
                    msg = edm.tile([128, TMAX, 128], FP8 if dr else DT, tag="msg")
                    nc.vector.tensor_tensor(out=msg[:, 0:nt, :],
                                            in0=ktb[:, 0:nt, :],
                                            in1=gt[:, 0:nt, H:2 * H], op=OP.mult)
                    pa = pedge.tile([128, 128], F32, tag="pa")
                    if dr:
                        npair = nt // 2
                        for t2 in range(0, npair * 2, 2):
                            nc.tensor.matmul(
                                out=pa[:], lhsT=msg[:, t2:t2 + 2, :],
                                rhs=bt[:, t2:t2 + 2, 0:128], start=(t2 == 0),
                                stop=(t2 + 2 == nt),
                                perf_mode=mybir.MatmulPerfMode.DoubleRow,
                                skip_group_check=True)
                        if nt % 2:
                            nc.tensor.matmul(out=pa[:], lhsT=msg[:, nt - 1, :],
                                             rhs=bt[:, nt - 1, 0:128],
                                             start=(nt == 1), stop=True,
                                             skip_group_check=True)
                    else:
                        for t in range(nt):
                            nc.tensor.matmul(out=pa[:], lhsT=msg[:, t, :],
                                             rhs=bt[:, t, 0:128], start=(t == 0),
                                             stop=(t == nt - 1))
                    nc.vector.tensor_tensor(
                        out=hs[:, g * 128:(g + 1) * 128],
                        in0=hs[:, g * 128:(g + 1) * 128], in1=pa[:], op=OP.add)
                    if after_group is not None:
                        after_group(g)

            # ---- layer-0 table from x: replicated input, so each rank builds
            # the FULL table locally (no collective, no exposed startup AG).
            def build_l0_chunk(c):
                g0, g1, tab, cbase = ((0, C0G, qv_tl[0][0], C0N) if c == 0
                                      else (C0G, NGRP, qv_tl[0][1], C1N))
                for rb in range(R):
                    g = g0
                    while g < g1:
                        ns = min(8, g1 - g)
                        lh = stg.tile([128, 1024], FP8, tag="lh")
                        nc.sync.dma_start(
                            lh[:, 0:ns * 128],
                            t_xrb[rb * 128:(rb + 1) * 128,
                                  g * 128:(g + ns) * 128])
                        st = stg.tile([128, 8, 2 * H], TDT, tag="st")
                        for s in range(0, ns, 2):
                            n2 = min(2, ns - s)
                            ps = pnode.tile([128, 512], F32, tag="pn")
                            for u in range(n2):
                                nc.tensor.matmul(
                                    out=ps[:, u * 256:u * 256 + 2 * H],
                                    lhsT=lh[:, (s + u) * 128:(s + u + 1) * 128],
                                    rhs=Wqvk_t[0][:, 0:2 * H], start=True,
                                    stop=not meta["has_bv"],
                                    skip_group_check=True)
                                if meta["has_bv"]:
                                    nc.tensor.matmul(
                                        out=ps[:, u * 256:u * 256 + 2 * H],
                                        lhsT=ones_t[:],
                                        rhs=bv_t[0][0:1, 0:2 * H],
                                        start=False, stop=True,
                                        skip_group_check=True)
                            if (s // 2) % 2 == 0:
                                nc.scalar.activation(st[:, s:s + n2, :],
                                                     ps[:, 0:n2 * 256], AF.Copy)
                            else:
                                nc.vector.tensor_copy(st[:, s:s + n2, :],
                                                      ps[:, 0:n2 * 256])
                        r0 = rb * cbase + (g - g0) * 128
                        nc.sync.dma_start(
                            tab[r0:r0 + ns * 128, :].rearrange(
                                "(s p) f -> p s f", p=128),
                            st[:, 0:ns, :])
                        g += ns
            build_l0_chunk(0)
            # local k table + s-part for layer 0 from the local x slice
            for g in range(NGRP):
                psk = pnode.tile([128, 512], F32, tag="pn")
                nc.tensor.matmul(out=psk[:, 0:H],
                                 lhsT=xT_t[:, g * 128:(g + 1) * 128],
                                 rhs=Wqvk_t[0][:, 2 * H:3 * H],
                                 start=True, stop=True, skip_group_check=True)
                nc.vector.tensor_copy(k_pp[0][:, g, :], psk[:, 0:H])
            emit_spart(0, xT_t)

            for _ in range(5):
                gz = edg.tile([128, TMAX, 2 * H], TDT, tag="g")
                nc.vector.memset(gz[:], 0.0)

            pp = ppool.tile([G, H], F32)
            indc = consts.tile([128, NGRP, G], DT)
            nc.sync.dma_start(
                indc[:], t_IndT[:].rearrange("(c p) g -> p c g", p=128))

            def make_pool_cb(l):
                def cb(g, l=l):
                    sl = slice(g * 128, (g + 1) * 128)
                    nc.vector.scalar_tensor_tensor(
                        out=h3_pp[l % 2][:, sl], in0=hs_pp[l % 2][:, sl],
                        scalar=float(SLOPE), in1=hs_pp[l % 2][:, sl],
                        op0=OP.mult, op1=OP.max)
                    trp = pedge.tile([128, 128], DT, tag="pa", name="trp_cb")
                    nc.tensor.transpose(out=trp[:], in_=h3f[:, sl],
                                        identity=ident[:])
                    hnode = tail.tile([128, 128], DT, tag="hnode",
                                      name="hnode_cb")
                    nc.vector.tensor_copy(hnode[:], trp[:])
                    nc.tensor.matmul(out=pp[:], lhsT=indc[:, g, :],
                                     rhs=hnode[:],
                                     start=(g == 0), stop=(g == NGRP - 1))
                return cb

            def emit_pool_part(c0g, c1g):
                for c in range(c0g, c1g):
                    trp = pedge.tile([128, 128], DT, tag="pa")
                    nc.tensor.transpose(out=trp[:],
                                        in_=h3f[:, c * 128:(c + 1) * 128],
                                        identity=ident[:])
                    hnode = tail.tile([128, 128], DT, tag="hnode")
                    nc.vector.tensor_copy(hnode[:], trp[:])
                    nc.tensor.matmul(out=pp[:],
                                     lhsT=indc[:, c, :],
                                     rhs=hnode[:],
                                     start=(c == 0), stop=(c == NGRP - 1))

            for l in range(NLAYER):
                hsrc_next = h3_pp[l % 2]
                last = l >= NLAYER - 1
                if parts >= 3 and not last:
                    cb0 = make_tail_cb(l, 0, C0G)
                    cb1 = make_tail_cb(l, C0G, NGRP)
                elif parts >= 4 and last:
                    cb0 = cb1 = make_pool_cb(l)
                else:
                    cb0 = cb1 = None
                if parts >= 2:
                    if l == 0:
                        # layer 0 consumes no AllGather (local tables), so run
                        # both epochs' first halves up front and fire the hook
                        # at ~50% -- the next layer's AG0 starts much earlier.
                        emit_edges(0, l, 0, 12)
                        build_l0_chunk(1)
                        emit_edges(0, l, 12, C0G)
                        emit_edges(1, l, 0, C0G, after_group=cb0)
                    else:
                        # full epoch-0 sweep first (its table is ready), then
                        # the first ep1 half; this keeps ep0 work ahead of any
                        # ep1 stall on the second table chunk's AllGather.
                        emit_edges(0, l, 0, NGRP)
                        emit_edges(1, l, 0, C0G, after_group=cb0)
                # h3 chunk-0 final: fire next layer's chunk-0 AllGather (the
                # per-group callback already built and stored the rows)
                if not last:
                    if parts < 2:
                        emit_leaky(l, 0)
                        build_chunk(l + 1, 0, hsrc_next)
                    emit_ag(l + 1, 0)
                elif cb0 is None:
                    emit_leaky(l, 0)
                if parts >= 2:
                    if l == 0:
                        emit_edges(0, l, C0G, NGRP)
                    emit_edges(1, l, C0G, NGRP, after_group=cb1)
                if parts < 3:
                    break
                if not last:
                    emit_ag(l + 1, 1)
                    emit_spart(l + 1, hsrc_next)
                elif cb1 is None:
                    emit_leaky(l, 1)
            if parts < 4:
                z_dbg = tail.tile([G, NCLS], F32, tag="zsb")
                nc.vector.tensor_copy(z_dbg[:], hs_pp[0][0:G, 0:NCLS])
                nc.sync.dma_start(t_out[:], z_dbg[:])
            else:
                pool_sb = tail.tile([G, H], F32, tag="poolsb")
                nc.vector.tensor_copy(pool_sb[:], pp[:])
                nc.sync.dma_start(pool_in[:], pool_sb[:])
                nc.gpsimd.collective_compute(
                    "AllGather", OP.bypass, replica_groups=[list(range(R))],
                    ins=[pool_in[:]], outs=[pool_out[:]])
                pr = tail.tile([G, R, H], F32, tag="pr")
                nc.sync.dma_start(pr[:], pool_out[:].rearrange("(r g) h -> g r h", r=R))
                pooled = tail.tile([G, H], F32, tag="pooled")
                nc.vector.tensor_tensor(out=pooled[:], in0=pr[:, 0, :], in1=pr[:, 1, :],
                                        op=OP.add)
                for r in range(2, R):
                    nc.vector.tensor_tensor(out=pooled[:], in0=pooled[:],
                                            in1=pr[:, r, :], op=OP.add)
                ptp = pedge.tile([H, G], F32, tag="pa")
                nc.tensor.transpose(out=ptp[:], in_=pooled[:], identity=identf[0:G, 0:G])
                pooledT = tail.tile([H, G], F32, tag="pooledT")
                nc.vector.tensor_copy(pooledT[:], ptp[:])
                zp = pedge.tile([G, NCLS], F32, tag="pa")
                nc.tensor.matmul(out=zp[:], lhsT=pooledT[:], rhs=Wch_t[:],
                                 start=True, stop=False)
                nc.tensor.matmul(out=zp[:], lhsT=clin_t[:], rhs=Wcc_t[:],
                                 start=False, stop=True)
                z_sb = tail.tile([G, NCLS], F32, tag="zsb")
                nc.vector.tensor_tensor(out=z_sb[:], in0=zp[:], in1=bc_t[:],
                                        op=OP.add)
                nc.sync.dma_start(t_out[:], z_sb[:])

    nc.compile()
    return nc


# ---------------------------------------------------------------------------

_CACHE = {}


def kernel(**inputs):
    in_maps, meta = prep(inputs)
    key = tuple(sorted((k, v) for k, v in meta.items()))
    if key not in _CACHE:
        _CACHE[key] = build(meta)
    nc = _CACHE[key]
    res = run_bass_kernel_spmd(nc, in_maps, list(range(R)))
    return np.asarray(res.results[0]["out"], np.float32)


def kernel_profiled(**inputs):
    """Like kernel() but also returns (exec_time_ns, trace_path)."""
    in_maps, meta = prep(inputs)
    key = tuple(sorted((k, v) for k, v in meta.items()))
    if key not in _CACHE:
        _CACHE[key] = build(meta)
    nc = _CACHE[key]
    res = run_bass_kernel_spmd(nc, in_maps, list(range(R)), trace=True)
    out = np.asarray(res.results[0]["out"], np.float32)
    trace_path = None
    if res.instructions_and_trace is not None:
        trace_path = res.instructions_and_trace[1]
    return out, res.exec_time_ns, trace_path


if __name__ == "__main__":
    pass
